# revision 1
# baseline (speedup 1.0000x reference)
"""DRC layer (dynamic range compressor) Trainium2 Bass kernel.

Problem: per batch row, y = x * 10^(-y_L/20) * 10^(mk/20) where y_L is a
branching one-pole smoother (attack/release) over the static gain curve
x_L computed in dB domain.  The smoother y[n] = a*y[n-1] + (1-a)*x_L[n]
with a in {alpha_A, alpha_R} chosen by (x_L[n] > y[n-1]) is solved by a
fixed-point iteration: guess y -> branch decisions -> the recurrence is
linear -> solve exactly with the hardware tensor_tensor_scan -> repeat.
In v = x_L - y space the recurrence is v[n] = a[n]*(v[n-1] - delta[n]),
delta[n] = x_L[n-1]-x_L[n], which is exactly one scan op
(state = (negdelta + state) * a).  Cross-chunk carries are solved exactly
each sweep with a tiny transposed scan over per-chunk affine maps
(A = prod a = exp(lR*L + (lA-lR)*sum d), f = v_end - A*v_init).
Schedule: 6 sweeps with one Aitken-extrapolated sweep (auto gamma from
boundary-delta ratios), then a final re-scan with exact carries.

Sharding: data-parallel, 2 batch rows per core x 8 cores.  Each core
packs its 2 rows as [128, 8192] (partitions 0-63 = row 0 in 64 chunks of
8192 samples, 64-127 = row 1).
"""
import sys
import numpy as np

try:
    from concourse import bass, bacc, mybir
except Exception:  # pragma: no cover
    for p in ("/opt/trn_rl_repo", "/root/.axon_site/_ro/trn_rl_repo"):
        if p not in sys.path:
            sys.path.insert(0, p)
    from concourse import bass, bacc, mybir

from concourse.bass_utils import run_bass_kernel_spmd
from concourse.tile import TileContext

f32 = np.float32
dt = mybir.dt
Op = mybir.AluOpType
Act = mybir.ActivationFunctionType

SR = f32(44100.0)
LOG9 = float(np.log(9.0))
CL = f32(20.0 / np.log(10.0))       # ln -> dB scale
RGAIN = f32(np.log(10.0) / 20.0)    # dB -> ln scale
P = 128                             # partitions
ROWS_PER_CORE = 2
N_CORES = 8
SCHED = "IIIEII"                    # I=sweep, E=extrapolated sweep
NCONST = 15


def host_consts(params, maxabs=None):
    """params [R,6] float32 -> per-row constants [R, NCONST] float32.
    Mirrors the reference's float32 arithmetic for the alphas."""
    p = params.astype(f32)
    p = np.where(np.isnan(p), f32(0.0), p)
    p = np.where(p == 0, f32(1e-10), p)
    T = (-p[:, 0] * f32(60.0)).astype(f32)
    ratio = (p[:, 1] * f32(10.0)).astype(f32)
    attack = np.maximum((p[:, 2] / f32(10.0)).astype(f32), f32(1e-4))
    release = np.maximum((p[:, 3] * f32(3.0)).astype(f32), f32(0.005))
    W = (p[:, 4] * f32(24.0)).astype(f32)
    mk = (p[:, 5] * f32(20.0)).astype(f32)
    aA = np.exp((f32(-LOG9) / (SR * attack)).astype(f32)).astype(f32)
    aR = np.exp((f32(-LOG9) / (SR * release)).astype(f32)).astype(f32)
    # derived (host f64 where it only affects our solver internals)
    lA = np.log(aA.astype(np.float64))
    lR = np.log(aR.astype(np.float64))
    c1 = (1.0 - 1.0 / ratio.astype(np.float64)).astype(f32)
    negc2 = (-1.0 / (8.0 * W.astype(np.float64) * ratio.astype(np.float64))).astype(f32)
    CL64 = np.float64(20.0 / np.log(10.0))
    T64 = T.astype(np.float64); W64 = W.astype(np.float64)
    out = np.zeros((p.shape[0], NCONST), f32)
    out[:, 0] = (-c1.astype(np.float64) * T64).astype(f32)   # negc1T
    out[:, 1] = (c1.astype(np.float64) * CL64).astype(f32)   # c1CL
    out[:, 2] = negc2
    out[:, 3] = ((W64 / 2 + T64) / CL64).astype(f32)         # thr_above (on ln)
    out[:, 4] = ((T64 - W64 / 2) / CL64).astype(f32)         # thr_below (on ln)
    out[:, 5] = (W64 - T64).astype(f32)                      # W - T (square bias)
    out[:, 6] = aR
    out[:, 7] = aA - aR               # dA
    out[:, 8] = 0.0                   # lRL: filled per-L at call site
    out[:, 9] = (lA - lR).astype(f32)  # dal
    out[:, 10] = (mk.astype(np.float64) * np.log(10.0) / 20.0).astype(f32)  # expbias
    out[:, 11] = 1e-8                 # eps for log
    dA = (aA - aR).astype(np.float64)
    dA = np.where(dA == 0, 1e-30, dA)
    out[:, 12] = ((1.0 - aR.astype(np.float64)) / dA).astype(f32)  # dstar
    # env warm start: aW = aR for fast-attack rows else 0; B = x_L lower bound
    out[:, 13] = np.where(aA < f32(0.99), aR, f32(0.0)).astype(f32)
    if maxabs is None:
        maxabs = np.full(p.shape[0], 1e4)
    uhi = 20.0 * np.log10(np.asarray(maxabs, np.float64) + 1e-8) - T64 + 1.0
    knee_min = -2.25 * (W64 ** 2) / (8.0 * W64 * ratio.astype(np.float64))
    c1f = c1.astype(np.float64)
    B = np.minimum(0.0, knee_min)
    B = np.minimum(B, np.where(c1f < 0, c1f * np.maximum(uhi, 0.0), 0.0)) - 1.0
    out[:, 14] = (-B).astype(f32)                            # negB
    out_lR = lR.astype(f32)
    return out, out_lR


def build_program(L):
    """Build the SPMD Bass program for chunk length L (8192 for the real
    problem). Returns the compiled Bacc."""
    nc = bacc.Bacc("TRN2", target_bir_lowering=False, debug=False,
                   num_devices=N_CORES)
    x_in = nc.dram_tensor("x", (P, L), dt.float32, kind="ExternalInput")
    cst_in = nc.dram_tensor("cst", (P, NCONST), dt.float32, kind="ExternalInput")
    aux_in = nc.dram_tensor("aux", (5, P), dt.float32, kind="ExternalInput")
    ident_in = nc.dram_tensor("ident", (P, P), dt.float32, kind="ExternalInput")
    y_out = nc.dram_tensor("y", (P, L), dt.float32, kind="ExternalOutput")

    v = nc.vector
    s = nc.scalar
    g = nc.gpsimd
    te = nc.tensor

    NB = 4                      # col blocks for iter/post pipelining
    LB = L // NB
    NBP = 8                     # pre col blocks
    LBP = L // NBP

    with TileContext(nc) as tc:
        with (
            tc.tile_pool(name="big", bufs=1) as big,
            tc.tile_pool(name="sm", bufs=2) as sm,
            tc.tile_pool(name="smk", bufs=4) as smk,
            tc.tile_pool(name="ps", bufs=1, space="PSUM") as ps,
            tc.tile_pool(name="dram", bufs=1, space="DRAM") as dram,
        ):
            # ---- persistent small tiles
            cst = sm.tile([P, NCONST], dt.float32, tag="cst")
            nc.sync.dma_start(out=cst[:], in_=cst_in[:])
            maskt = sm.tile([1, P], dt.float32, tag="maskt")
            nc.sync.dma_start(out=maskt[:], in_=aux_in[0:1, :])
            mtt = sm.tile([2, P], dt.float32, tag="mtt")
            nc.sync.dma_start(out=mtt[:], in_=aux_in[1:3, :])
            onest = sm.tile([1, 1], dt.float32, tag="onest")
            nc.sync.dma_start(out=onest[:], in_=aux_in[3:4, 0:1])
            alwt = sm.tile([1, P], dt.float32, tag="alwt")
            nc.sync.dma_start(out=alwt[:], in_=aux_in[4:5, :])
            ident = sm.tile([P, P], dt.float32, tag="ident")
            nc.sync.dma_start(out=ident[:], in_=ident_in[:])
            startmask = maskt[0:1, :]    # [1,128]: 0 at chunk 0 and 64
            mt = mtt[0:2, :]             # [2,128] row-block indicator
            ones11 = onest[0:1, 0:1]     # [1,1] = 1.0

            def col(i):
                return cst[:, i:i + 1]

            # ---- big slots S1..S5 (32KB/partition each)
            S1 = big.tile([P, L], dt.float32, tag="S1")  # ND
            S2 = big.tile([P, L], dt.float32, tag="S2")  # x -> temps -> D
            S3 = big.tile([P, L], dt.float32, tag="S3")  # XL -> (spill) Vtmp
            S4 = big.tile([P, L], dt.float32, tag="S4")  # V
            S5 = big.tile([P, L], dt.float32, tag="S5")  # d/a
            ma32 = big.tile([P, L], dt.int32, tag="S4")  # PRE-only alias of S4
            spill = dram.tile([P, L], dt.float32, tag="spill")

            # ================= PRE: x -> x_L, D, ND (col-blocked) ========
            for b in range(NBP):
                sl = slice(b * LBP, (b + 1) * LBP)
                nc.sync.dma_start(out=S2[:, sl], in_=x_in[:, sl])
                # ACT: abs -> ln -> (CL*ln + (W-T))^2 -> knee; DVE off ln
                s.activation(S1[:, sl], S2[:, sl], Act.Abs, bias=0.0, scale=1.0)
                s.activation(S2[:, sl], S1[:, sl], Act.Ln, bias=col(11), scale=1.0)
                s.activation(S1[:, sl], S2[:, sl], Act.Square, bias=col(5),
                             scale=float(CL))
                v.tensor_scalar(out=S3[:, sl], in0=S1[:, sl], scalar1=col(2),
                                scalar2=None, op0=Op.mult)
                v.tensor_scalar(out=S5[:, sl], in0=S2[:, sl], scalar1=col(1),
                                scalar2=col(0), op0=Op.mult, op1=Op.add)
                v.tensor_scalar(out=ma32[:, sl], in0=S2[:, sl], scalar1=col(3),
                                scalar2=None, op0=Op.is_gt)
                v.copy_predicated(S3[:, sl], ma32[:, sl], S5[:, sl])
                v.tensor_scalar(out=S5[:, sl], in0=S2[:, sl], scalar1=col(4),
                                scalar2=None, op0=Op.is_ge)
                v.tensor_tensor(out=S3[:, sl], in0=S3[:, sl], in1=S5[:, sl],
                                op=Op.mult)
                # S3[:, sl] = x_L block. delta into S2 (cols shifted by 1)
                lo = b * LBP
                hi = (b + 1) * LBP
                v.tensor_tensor(out=S2[:, max(lo, 1):hi],
                                in0=S3[:, max(lo, 1) - 1:hi - 1],
                                in1=S3[:, max(lo, 1):hi], op=Op.subtract)
                s.activation(S1[:, max(lo, 1):hi], S2[:, max(lo, 1):hi],
                             Act.Identity, bias=0.0, scale=-1.0)
                nc.sync.dma_start(out=spill[:, sl], in_=S3[:, sl])
            # cross-chunk delta col 0: prevlast[p] = x_L[p-1, L-1], rows reset 0
            pl = smk.tile([P, 1], dt.float32, tag="pl")
            v.memset(pl[:], 0.0)
            nc.sync.dma_start(out=pl[1:P, :], in_=S3[0:P - 1, L - 1:L])
            v.memset(pl[64:65, :], 0.0)
            v.memset(pl[0:1, :], 0.0)
            v.tensor_tensor(out=S2[:, 0:1], in0=pl[:], in1=S3[:, 0:1],
                            op=Op.subtract)
            v.tensor_scalar(out=S1[:, 0:1], in0=S2[:, 0:1], scalar1=-1.0,
                            scalar2=None, op0=Op.mult)

            # ================= iteration machinery =================
            def boundary_A(vinit_used, sd, bias_ap):
                """A-column and A*vinit: only needs sum(d); overlaps scans."""
                logA = smk.tile([P, 1], dt.float32, tag="logA")
                v.scalar_tensor_tensor(out=logA[:], in0=sd, scalar=col(9),
                                       in1=bias_ap, op0=Op.mult, op1=Op.add)
                A_c = smk.tile([P, 1], dt.float32, tag="A_c")
                s.activation(A_c[:], logA[:], Act.Exp, bias=0.0, scale=1.0)
                t1 = smk.tile([P, 1], dt.float32, tag="t1")
                if vinit_used is None:
                    v.memset(t1[:], 0.0)
                else:
                    v.tensor_tensor(out=t1[:], in0=A_c[:], in1=vinit_used,
                                    op=Op.mult)
                return A_c, t1

            def boundary_chain(V_t, A_c, t1):
                f_c = smk.tile([P, 1], dt.float32, tag="f_c")
                v.tensor_tensor(out=f_c[:], in0=V_t[:, L - 1:L], in1=t1[:],
                                op=Op.subtract)
                ap_p = ps.tile([1, P], dt.float32, tag="ap_p")
                te.transpose(ap_p[:], A_c[:], ident[:])
                a_row = smk.tile([1, P], dt.float32, tag="a_row")
                v.tensor_tensor(out=a_row[:], in0=ap_p[:], in1=startmask,
                                op=Op.mult)
                fp_p = ps.tile([1, P], dt.float32, tag="fp_p")
                te.transpose(fp_p[:], f_c[:], ident[:])
                f_row = smk.tile([1, P], dt.float32, tag="f_row")
                v.tensor_copy(f_row[:], fp_p[:])
                zr = smk.tile([1, P], dt.float32, tag="zr")
                v.tensor_tensor_scan(zr[:], a_row[:], f_row[:], 0.0,
                                     Op.mult, Op.add)
                zs = smk.tile([1, P], dt.float32, tag="zs")
                v.memset(zs[:], 0.0)
                v.tensor_copy(zs[0:1, 1:P], zr[0:1, 0:P - 1])
                v.tensor_tensor(out=zs[:], in0=zs[:], in1=startmask, op=Op.mult)
                vip = ps.tile([P, 1], dt.float32, tag="vip")
                te.transpose(vip[:], zs[:], ones11)
                vic = smk.tile([P, 1], dt.float32, tag="vic")
                v.tensor_copy(vic[:], vip[:])
                return vic, zs

            def row_broadcast(pair_row):
                pr = ps.tile([2, 1], dt.float32, tag="pr")
                te.transpose(pr[:], pair_row, ones11)
                prs = smk.tile([2, 1], dt.float32, tag="prs")
                v.tensor_copy(prs[:], pr[:])
                cb = ps.tile([P, 1], dt.float32, tag="cb")
                te.matmul(cb[:], mt, prs[:])
                out = smk.tile([P, 1], dt.float32, tag="bc")
                v.tensor_copy(out[:], cb[:])
                return out

            # ---------- env warm start (2-pass shifted decaying-max) ------
            # xh = x_L - B into S4 ; aW tile into S5 ; env scans into S3
            v.tensor_scalar(out=S4[:], in0=S3[:], scalar1=col(14), scalar2=None,
                            op0=Op.add)
            v.tensor_scalar(out=S5[:], in0=S4[:], scalar1=0.0, scalar2=col(13),
                            op0=Op.mult, op1=Op.add)
            v.tensor_tensor_scan(S3[:], S5[:], S4[:], 0.0, Op.mult, Op.max)
            # cross-chunk: z = maxscan(ALW, ends); env pass 2 init = shift(z)
            ep = ps.tile([1, P], dt.float32, tag="ap_p")
            te.transpose(ep[:], S3[:, L - 1:L], ident[:])
            e_row = smk.tile([1, P], dt.float32, tag="a_row")
            v.tensor_copy(e_row[:], ep[:])
            ze = smk.tile([1, P], dt.float32, tag="zr")
            v.tensor_tensor_scan(ze[:], alwt[0:1, :], e_row[:], 0.0,
                                 Op.mult, Op.max)
            zse = smk.tile([1, P], dt.float32, tag="zs")
            v.memset(zse[:], 0.0)
            v.tensor_copy(zse[0:1, 1:P], ze[0:1, 0:P - 1])
            v.tensor_tensor(out=zse[:], in0=zse[:], in1=startmask, op=Op.mult)
            evp = ps.tile([P, 1], dt.float32, tag="vip")
            te.transpose(evp[:], zse[:], ones11)
            env0 = smk.tile([P, 1], dt.float32, tag="vic")
            v.tensor_copy(env0[:], evp[:])
            v.tensor_tensor_scan(S3[:], S5[:], S4[:], env0[:], Op.mult, Op.max)
            # v0 = xh - env into S4 ; vinit0 = shift(xh_end - env_end)
            vz = smk.tile([P, 1], dt.float32, tag="t1")
            v.tensor_tensor(out=vz[:], in0=S4[:, L - 1:L], in1=S3[:, L - 1:L],
                            op=Op.subtract)
            v.tensor_tensor(out=S4[:], in0=S4[:], in1=S3[:], op=Op.subtract)
            vzp = ps.tile([1, P], dt.float32, tag="fp_p")
            te.transpose(vzp[:], vz[:], ident[:])
            vz_row = smk.tile([1, P], dt.float32, tag="f_row")
            v.tensor_copy(vz_row[:], vzp[:])
            vzs = smk.tile([1, P], dt.float32, tag="d1")
            v.memset(vzs[:], 0.0)
            v.tensor_copy(vzs[0:1, 1:P], vz_row[0:1, 0:P - 1])
            v.tensor_tensor(out=vzs[:], in0=vzs[:], in1=startmask, op=Op.mult)
            vip0 = ps.tile([P, 1], dt.float32, tag="vip")
            te.transpose(vip0[:], vzs[:], ones11)
            vinit0 = smk.tile([P, 1], dt.float32, tag="vic")
            v.tensor_copy(vinit0[:], vip0[:])

            vinit = vinit0
            vinit_prev = None
            zrow_hist = [vzs]
            V_cur = S4

            for k, step in enumerate(SCHED):
                sd = smk.tile([P, 1], dt.float32, tag="sd")
                lbias = col(8)  # lRL
                if False:
                    pass
                else:
                    Vp = V_cur
                    vic_used = vinit
                    if step == "E":
                        z0, z1, z2 = zrow_hist[-1], zrow_hist[-2], zrow_hist[-3]
                        d1 = smk.tile([1, P], dt.float32, tag="d1")
                        v.tensor_tensor(out=d1[:], in0=z0[:], in1=z1[:], op=Op.subtract)
                        s.activation(d1[:], d1[:], Act.Abs, bias=0.0, scale=1.0)
                        d2 = smk.tile([1, P], dt.float32, tag="d2")
                        v.tensor_tensor(out=d2[:], in0=z1[:], in1=z2[:], op=Op.subtract)
                        s.activation(d2[:], d2[:], Act.Abs, bias=0.0, scale=1.0)
                        rs = smk.tile([1, 2], dt.float32, tag="rs")
                        rs2 = smk.tile([1, 2], dt.float32, tag="rs2")
                        half = P // 2
                        v.tensor_reduce(rs[0:1, 0:1], d1[0:1, 0:half], mybir.AxisListType.X, Op.add)
                        v.tensor_reduce(rs[0:1, 1:2], d1[0:1, half:P], mybir.AxisListType.X, Op.add)
                        v.tensor_reduce(rs2[0:1, 0:1], d2[0:1, 0:half], mybir.AxisListType.X, Op.add)
                        v.tensor_reduce(rs2[0:1, 1:2], d2[0:1, half:P], mybir.AxisListType.X, Op.add)
                        v.tensor_scalar(out=rs2[:], in0=rs2[:], scalar1=1e-30,
                                        scalar2=None, op0=Op.add)
                        rho = smk.tile([1, 2], dt.float32, tag="rho")
                        v.reciprocal(rs2[:], rs2[:])
                        v.tensor_tensor(out=rho[:], in0=rs[:], in1=rs2[:], op=Op.mult)
                        v.tensor_scalar(out=rho[:], in0=rho[:], scalar1=0.95,
                                        scalar2=None, op0=Op.min)
                        om = smk.tile([1, 2], dt.float32, tag="om")
                        v.tensor_scalar(out=om[:], in0=rho[:], scalar1=-1.0,
                                        scalar2=1.0, op0=Op.mult, op1=Op.add)
                        v.reciprocal(om[:], om[:])
                        gam = smk.tile([1, 2], dt.float32, tag="gam")
                        v.tensor_tensor(out=gam[:], in0=rho[:], in1=om[:], op=Op.mult)
                        gcol = row_broadcast(gam[:])
                        gp1 = smk.tile([P, 1], dt.float32, tag="gp1")
                        v.tensor_scalar(out=gp1[:], in0=gcol[:], scalar1=1.0,
                                        scalar2=None, op0=Op.add)
                        Vm1 = S4 if V_cur is S3 else S3
                        v.tensor_scalar(out=Vm1[:], in0=Vm1[:], scalar1=gcol[:],
                                        scalar2=None, op0=Op.mult)
                        v.scalar_tensor_tensor(out=Vm1[:], in0=Vp[:],
                                               scalar=gp1[:], in1=Vm1[:],
                                               op0=Op.mult, op1=Op.subtract)
                        Vp = Vm1
                        dv = smk.tile([P, 1], dt.float32, tag="dv")
                        v.tensor_tensor(out=dv[:], in0=vinit[:], in1=vinit_prev[:],
                                        op=Op.subtract)
                        vice = smk.tile([P, 1], dt.float32, tag="vice")
                        v.scalar_tensor_tensor(out=vice[:], in0=dv[:], scalar=gcol[:],
                                               in1=vinit[:], op0=Op.mult, op1=Op.add)
                        vic_used = vice
                    # blocked d with per-block partial sums (stt bypass/is_gt)
                    sdb = smk.tile([P, NB], dt.float32, tag="sdb")
                    v.tensor_tensor(out=S5[:, 0:1], in0=vic_used[:], in1=S2[:, 0:1],
                                    op=Op.is_gt)
                    for b in range(NB):
                        lo = b * LB
                        hi = (b + 1) * LB
                        l2 = max(lo, 1)
                        v.scalar_tensor_tensor(out=S5[:, l2:hi],
                                               in0=Vp[:, l2 - 1:hi - 1],
                                               scalar=1.0, in1=S2[:, l2:hi],
                                               op0=Op.bypass, op1=Op.is_gt,
                                               accum_out=sdb[:, b:b + 1])
                    v.tensor_reduce(sd[:], sdb[:], mybir.AxisListType.X, Op.add)
                    # fold col-0 decision into the logA bias
                    tl = smk.tile([P, 1], dt.float32, tag="tl")
                    v.tensor_scalar(out=tl[:], in0=S5[:, 0:1], scalar1=col(9),
                                    scalar2=col(8), op0=Op.mult, op1=Op.add)
                    lbias = tl[:]
                    vinit_used = vic_used
                A_c, t1 = boundary_A(
                    None if vinit_used is None else vinit_used[:], sd[:], lbias)
                # a = dA*d + aR (ACT, hidden under the scan chain), then
                # row-start a[.,0]=1, and chained block scans
                Vout = S3 if k == 2 else S4
                a_dst = S3 if k == 0 else S5
                if k > 0:
                    v.tensor_copy(S5[0:1, 0:1], cst[0:1, 12:13])
                    v.tensor_copy(S5[64:65, 0:1], cst[64:65, 12:13])
                for b in range(NB):
                    lo = b * LB
                    hi = (b + 1) * LB
                    if b == 0:
                        v.tensor_scalar(out=a_dst[:, lo:hi], in0=S5[:, lo:hi],
                                        scalar1=col(7), scalar2=col(6),
                                        op0=Op.mult, op1=Op.add)
                    else:
                        s.activation(a_dst[:, lo:hi], S5[:, lo:hi], Act.Identity,
                                     bias=col(6), scale=col(7))
                    if b == 0:
                        init_ap = 0.0 if vinit_used is None else vinit_used[:]
                    else:
                        init_ap = Vout[:, lo - 1:lo]
                    v.tensor_tensor_scan(Vout[:, lo:hi], S1[:, lo:hi],
                                         a_dst[:, lo:hi], init_ap,
                                         Op.add, Op.mult)
                V_cur = Vout
                vinit_prev = vinit
                vic, zs = boundary_chain(Vout, A_c, t1)
                vinit = vic
                zrow_hist.append(zs)
                if len(zrow_hist) > 3:
                    zrow_hist.pop(0)

            # final exact re-scan (blocked) with POST chained per block
            nc.sync.dma_start(out=S3[:], in_=spill[:])
            nc.sync.dma_start(out=S2[:], in_=x_in[:])
            for b in range(NB):
                lo = b * LB
                hi = (b + 1) * LB
                sl = slice(lo, hi)
                init_ap = vinit[:] if b == 0 else S4[:, lo - 1:lo]
                v.tensor_tensor_scan(S4[:, sl], S1[:, sl], S5[:, sl], init_ap,
                                     Op.add, Op.mult)
                v.tensor_tensor(out=S5[:, sl], in0=S4[:, sl], in1=S3[:, sl],
                                op=Op.subtract)
                v.tensor_scalar(out=S5[:, sl], in0=S5[:, sl], scalar1=96.0,
                                scalar2=None, op0=Op.min)
                s.activation(S1[:, sl], S5[:, sl], Act.Exp, bias=col(10),
                             scale=float(RGAIN))
                v.tensor_tensor(out=S3[:, sl], in0=S1[:, sl], in1=S2[:, sl],
                                op=Op.mult)
                nc.sync.dma_start(out=y_out[:, sl], in_=S3[:, sl])

    nc.compile()
    return nc


_CACHE = {}
PROFILE = False
LAST_EXEC_NS = None
LAST_RESULTS = None


def _get_program(L):
    if L not in _CACHE:
        _CACHE[L] = build_program(L)
    return _CACHE[L]


def make_core_inputs(x, params, L):
    """Full x [B,N], params [B,6] -> list of per-core input dicts."""
    B, N = x.shape
    n_cores = B // ROWS_PER_CORE
    rows_per_core = ROWS_PER_CORE
    maxabs = np.abs(x).max(axis=1)
    csts, lR = host_consts(params, maxabs)
    csts[:, 8] = (lR.astype(np.float64) * L).astype(f32)
    ident = np.eye(P, dtype=f32)
    in_maps = []
    for c in range(n_cores):
        rows = slice(c * rows_per_core, (c + 1) * rows_per_core)
        xs = np.ascontiguousarray(x[rows]).reshape(P, L)
        cc = np.repeat(csts[rows], P // rows_per_core, axis=0)
        aux = np.zeros((5, P), f32)
        aux[0, :] = 1.0
        aux[0, 0] = 0.0
        aux[0, 64] = 0.0
        aux[1, 0:64] = 1.0   # Mt row 0 -> partitions 0..63
        aux[2, 64:128] = 1.0
        aux[3, 0] = 1.0      # ones11
        # ALW row: aW^L per chunk, 0 at row starts
        aW2 = cc[:, 13].astype(np.float64)
        aux[4, :] = np.exp(np.log(np.maximum(aW2, 1e-300)) * L).astype(f32)
        aux[4, :] = np.where(aW2 == 0, 0.0, aux[4, :])
        aux[4, 0] = 0.0
        aux[4, 64] = 0.0
        in_maps.append(dict(x=xs, cst=np.ascontiguousarray(cc),
                            aux=aux, ident=ident))
    return in_maps


def kernel(x, params):
    x = np.asarray(x)
    params = np.asarray(params, f32)
    B, N = x.shape
    L = (N * ROWS_PER_CORE) // P
    nc = _get_program(L)
    in_maps = make_core_inputs(np.asarray(x, f32), params, L)
    global LAST_EXEC_NS, LAST_RESULTS
    res = run_bass_kernel_spmd(nc, in_maps, list(range(B // ROWS_PER_CORE)),
                               trace=PROFILE)
    LAST_EXEC_NS = res.exec_time_ns
    LAST_RESULTS = res
    outs = [r["y"].reshape(ROWS_PER_CORE, N) for r in res.results]
    return np.concatenate(outs, axis=0).astype(x.dtype, copy=False)



# revision 6
# speedup vs baseline: 1.2967x; 1.2967x over previous
"""DRC layer (dynamic range compressor) Trainium2 Bass kernel.

Per batch row, y = x * 10^(-y_L/20) * 10^(mk/20) where y_L is a branching
one-pole smoother (attack/release) over the static gain curve x_L
computed in dB domain.  Solved by fixed-point iteration: guess y ->
branch decisions -> recurrence is linear -> solve exactly with the
hardware tensor_tensor_scan -> repeat.  In v = x_L - y space the
recurrence is v[n] = a[n]*(v[n-1] - delta[n]), which is one scan op
(state = (negdelta + state) * a).  Cross-chunk carries are solved
exactly each sweep with a tiny transposed scan over per-chunk affine
maps.  Schedule: 3 sweeps + 1 Aitken-extrapolated sweep (gamma from
boundary-delta ratios), then a final re-scan with exact carries.

vs the 376981ns baseline: 2 fewer sweeps (IIIE vs IIIEII), no DRAM
spill of x_L (V_{k-2} shadow kept in fp16 so SBUF fits), env warm start
pass 1 dropped (chunk-local init), -96 clip moved to the Act engine
(Relu), x re-read by DMA during the last sweep into the dead D buffer.

Sharding: data-parallel, 2 batch rows per core x 8 cores.  Each core
packs its 2 rows as [128, 8192] (partitions 0-63 = row 0 in 64 chunks
of 8192 samples, 64-127 = row 1).
"""
import sys
import numpy as np

try:
    from concourse import bass, bacc, mybir
except Exception:  # pragma: no cover
    for p in ("/opt/trn_rl_repo", "/root/.axon_site/_ro/trn_rl_repo"):
        if p not in sys.path:
            sys.path.insert(0, p)
    from concourse import bass, bacc, mybir

from concourse.bass_utils import run_bass_kernel_spmd
from concourse.tile import TileContext

f32 = np.float32
dt = mybir.dt
Op = mybir.AluOpType
Act = mybir.ActivationFunctionType

SR = f32(44100.0)
LOG9 = float(np.log(9.0))
CL = f32(20.0 / np.log(10.0))       # ln -> dB scale
RGAIN = f32(np.log(10.0) / 20.0)    # dB -> ln scale
P = 128                             # partitions
ROWS_PER_CORE = 2
N_CORES = 8
SCHED = "IIIE"                      # I=sweep, E=extrapolated sweep
NCONST = 17


def host_consts(params, maxabs=None):
    """params [R,6] float32 -> per-row constants [R, NCONST] float32.
    Mirrors the reference's float32 arithmetic for the alphas."""
    p = params.astype(f32)
    p = np.where(np.isnan(p), f32(0.0), p)
    p = np.where(p == 0, f32(1e-10), p)
    T = (-p[:, 0] * f32(60.0)).astype(f32)
    ratio = (p[:, 1] * f32(10.0)).astype(f32)
    attack = np.maximum((p[:, 2] / f32(10.0)).astype(f32), f32(1e-4))
    release = np.maximum((p[:, 3] * f32(3.0)).astype(f32), f32(0.005))
    W = (p[:, 4] * f32(24.0)).astype(f32)
    mk = (p[:, 5] * f32(20.0)).astype(f32)
    aA = np.exp((f32(-LOG9) / (SR * attack)).astype(f32)).astype(f32)
    aR = np.exp((f32(-LOG9) / (SR * release)).astype(f32)).astype(f32)
    # derived (host f64 where it only affects our solver internals)
    lA = np.log(aA.astype(np.float64))
    lR = np.log(aR.astype(np.float64))
    c1 = (1.0 - 1.0 / ratio.astype(np.float64)).astype(f32)
    negc2 = (-1.0 / (8.0 * W.astype(np.float64) * ratio.astype(np.float64))).astype(f32)
    CL64 = np.float64(20.0 / np.log(10.0))
    T64 = T.astype(np.float64); W64 = W.astype(np.float64)
    out = np.zeros((p.shape[0], NCONST), f32)
    out[:, 0] = (-c1.astype(np.float64) * T64).astype(f32)   # negc1T
    out[:, 1] = (c1.astype(np.float64) * CL64).astype(f32)   # c1CL
    out[:, 2] = negc2
    out[:, 3] = ((W64 / 2 + T64) / CL64).astype(f32)         # thr_above (on ln)
    out[:, 4] = ((T64 - W64 / 2) / CL64).astype(f32)         # thr_below (on ln)
    out[:, 5] = (W64 - T64).astype(f32)                      # W - T (square bias)
    out[:, 6] = aR
    out[:, 7] = aA - aR               # dA
    out[:, 8] = 0.0                   # lRL: filled per-L at call site
    out[:, 9] = (lA - lR).astype(f32)  # dal
    out[:, 10] = (mk.astype(np.float64) * np.log(10.0) / 20.0).astype(f32)  # expbias
    out[:, 11] = 1e-8                 # eps for log
    dA = (aA - aR).astype(np.float64)
    dA = np.where(dA == 0, 1e-30, dA)
    out[:, 12] = ((1.0 - aR.astype(np.float64)) / dA).astype(f32)  # dstar
    # env warm start: aW = aR for fast-attack rows else 0; B = x_L lower bound
    out[:, 13] = np.where(aA < f32(0.99), aR, f32(0.0)).astype(f32)
    if maxabs is None:
        maxabs = np.full(p.shape[0], 1e4)
    uhi = 20.0 * np.log10(np.asarray(maxabs, np.float64) + 1e-8) - T64 + 1.0
    knee_min = -2.25 * (W64 ** 2) / (8.0 * W64 * ratio.astype(np.float64))
    c1f = c1.astype(np.float64)
    B = np.minimum(0.0, knee_min)
    B = np.minimum(B, np.where(c1f < 0, c1f * np.maximum(uhi, 0.0), 0.0)) - 1.0
    out[:, 14] = (-B).astype(f32)                            # negB
    # expbias + 96*RGAIN (post: exp(-RGAIN*relu(96-(-y_L)) + col15))
    out[:, 15] = (out[:, 10].astype(np.float64)
                  + 96.0 * np.log(10.0) / 20.0).astype(f32)
    out[:, 16] = 96.0                                        # relu bias
    out_lR = lR.astype(f32)
    return out, out_lR


def build_program(L):
    """Build the SPMD Bass program for chunk length L (8192 for the real
    problem). Returns the compiled Bacc."""
    nc = bacc.Bacc("TRN2", target_bir_lowering=False, debug=False,
                   num_devices=N_CORES)
    x_in = nc.dram_tensor("x", (P, L), dt.float32, kind="ExternalInput")
    cst_in = nc.dram_tensor("cst", (P, NCONST), dt.float32, kind="ExternalInput")
    aux_in = nc.dram_tensor("aux", (5, P), dt.float32, kind="ExternalInput")
    ident_in = nc.dram_tensor("ident", (P, P), dt.float32, kind="ExternalInput")
    y_out = nc.dram_tensor("y", (P, L), dt.float32, kind="ExternalOutput")

    v = nc.vector
    s = nc.scalar
    te = nc.tensor

    NB = 4                      # col blocks for iter/post pipelining
    LB = L // NB
    NBP = 4                     # pre col blocks
    LBP = L // NBP

    with TileContext(nc) as tc:
        with (
            tc.tile_pool(name="big", bufs=1) as big,
            tc.tile_pool(name="sm", bufs=2) as sm,
            tc.tile_pool(name="smk", bufs=4) as smk,
            tc.tile_pool(name="ps", bufs=1, space="PSUM") as ps,
        ):
            # ---- persistent small tiles
            cst = sm.tile([P, NCONST], dt.float32, tag="cst")
            nc.sync.dma_start(out=cst[:], in_=cst_in[:])
            maskt = sm.tile([1, P], dt.float32, tag="maskt")
            nc.sync.dma_start(out=maskt[:], in_=aux_in[0:1, :])
            mtt = sm.tile([2, P], dt.float32, tag="mtt")
            nc.sync.dma_start(out=mtt[:], in_=aux_in[1:3, :])
            onest = sm.tile([1, 1], dt.float32, tag="onest")
            nc.sync.dma_start(out=onest[:], in_=aux_in[3:4, 0:1])
            ident = sm.tile([P, P], dt.float32, tag="ident")
            nc.sync.dma_start(out=ident[:], in_=ident_in[:])
            startmask = maskt[0:1, :]    # [1,128]: 0 at chunk 0 and 64
            mt = mtt[0:2, :]             # [2,128] row-block indicator
            ones11 = onest[0:1, 0:1]     # [1,1] = 1.0

            def col(i):
                return cst[:, i:i + 1]

            # ---- big slots (32KB/partition each; F16 is 16KB)
            S1 = big.tile([P, L], dt.float32, tag="S1")  # ND
            S2 = big.tile([P, L], dt.float32, tag="S2")  # x -> D -> x again
            S3 = big.tile([P, L], dt.float32, tag="S3")  # x_L (never spilled)
            S4 = big.tile([P, L], dt.float32, tag="S4")  # xh -> V
            S5 = big.tile([P, L], dt.float32, tag="S5")  # scratch -> d/a
            F16 = big.tile([P, L], dt.float16, tag="F16")  # env, then Vm1 shadow
            ma32 = big.tile([P, L], dt.int32, tag="S4")   # PRE-only alias of S4

            # ================= PRE: x -> x_L, D, ND (col-blocked) ========
            # per block: x->S2; Act: Abs->S4, Ln->S2(u), Square->S4(sq),
            # lin->S5(Act affine); DVE: knee TS, is_gt mask(ma32=S5 alias..)
            # NOTE: mask lives in S4-alias?  S4 holds sq.  Order below keeps
            # each tile single-use-per-stage; ma32 aliases S5 only AFTER lin
            # has been consumed by copy_predicated.
            for b in range(NBP):
                sl = slice(b * LBP, (b + 1) * LBP)
                nc.sync.dma_start(out=S2[:, sl], in_=x_in[:, sl])
                s.activation(S1[:, sl], S2[:, sl], Act.Abs, bias=0.0, scale=1.0)
                s.activation(S2[:, sl], S1[:, sl], Act.Ln, bias=col(11), scale=1.0)
                # u = ln(|x|+eps) in S2.  knee = negc2*(CL*u + (W-T))^2
                s.activation(S1[:, sl], S2[:, sl], Act.Square, bias=col(5),
                             scale=float(CL))
                v.tensor_scalar(out=S3[:, sl], in0=S1[:, sl], scalar1=col(2),
                                scalar2=None, op0=Op.mult)
                # lin = c1CL*u + negc1T  (Act affine)
                s.activation(S5[:, sl], S2[:, sl], Act.Identity, bias=col(0),
                             scale=col(1))
                # above mask -> S4 (int alias view is S5-tagged; use direct
                # f32 compare into S4 then copy_predicated needs int mask ->
                # reuse baseline trick: write compare into ma32 (S5 alias)
                # would clobber lin.  Instead compare into S4 as int32 alias.
                v.tensor_scalar(out=ma32[:, sl], in0=S2[:, sl], scalar1=col(3),
                                scalar2=None, op0=Op.is_gt)
                v.copy_predicated(S3[:, sl], ma32[:, sl], S5[:, sl])
                # below mask zeroing
                v.tensor_scalar(out=S5[:, sl], in0=S2[:, sl], scalar1=col(4),
                                scalar2=None, op0=Op.is_ge)
                v.tensor_tensor(out=S3[:, sl], in0=S3[:, sl], in1=S5[:, sl],
                                op=Op.mult)
                # S3[:, sl] = x_L block. delta into S2 (cols shifted by 1)
                lo = b * LBP
                hi = (b + 1) * LBP
                v.tensor_tensor(out=S2[:, max(lo, 1):hi],
                                in0=S3[:, max(lo, 1) - 1:hi - 1],
                                in1=S3[:, max(lo, 1):hi], op=Op.subtract)
                s.activation(S1[:, max(lo, 1):hi], S2[:, max(lo, 1):hi],
                             Act.Identity, bias=0.0, scale=-1.0)
            # cross-chunk delta col 0: prevlast[p] = x_L[p-1, L-1], rows reset 0
            pl = smk.tile([P, 1], dt.float32, tag="pl")
            v.memset(pl[:], 0.0)
            nc.sync.dma_start(out=pl[1:P, :], in_=S3[0:P - 1, L - 1:L])
            v.memset(pl[64:65, :], 0.0)
            v.memset(pl[0:1, :], 0.0)
            v.tensor_tensor(out=S2[:, 0:1], in0=pl[:], in1=S3[:, 0:1],
                            op=Op.subtract)
            v.tensor_scalar(out=S1[:, 0:1], in0=S2[:, 0:1], scalar1=-1.0,
                            scalar2=None, op0=Op.mult)

            # ================= iteration machinery =================
            def boundary_A(vinit_used, sd, bias_ap):
                """A-column and A*vinit: only needs sum(d); overlaps scans."""
                logA = smk.tile([P, 1], dt.float32, tag="logA")
                v.scalar_tensor_tensor(out=logA[:], in0=sd, scalar=col(9),
                                       in1=bias_ap, op0=Op.mult, op1=Op.add)
                A_c = smk.tile([P, 1], dt.float32, tag="A_c")
                s.activation(A_c[:], logA[:], Act.Exp, bias=0.0, scale=1.0)
                t1 = smk.tile([P, 1], dt.float32, tag="t1")
                if vinit_used is None:
                    v.memset(t1[:], 0.0)
                else:
                    v.tensor_tensor(out=t1[:], in0=A_c[:], in1=vinit_used,
                                    op=Op.mult)
                return A_c, t1

            def boundary_chain(V_t, A_c, t1):
                f_c = smk.tile([P, 1], dt.float32, tag="f_c")
                v.tensor_tensor(out=f_c[:], in0=V_t[:, L - 1:L], in1=t1[:],
                                op=Op.subtract)
                ap_p = ps.tile([1, P], dt.float32, tag="ap_p")
                te.transpose(ap_p[:], A_c[:], ident[:])
                a_row = smk.tile([1, P], dt.float32, tag="a_row")
                v.tensor_tensor(out=a_row[:], in0=ap_p[:], in1=startmask,
                                op=Op.mult)
                fp_p = ps.tile([1, P], dt.float32, tag="fp_p")
                te.transpose(fp_p[:], f_c[:], ident[:])
                f_row = smk.tile([1, P], dt.float32, tag="f_row")
                v.tensor_copy(f_row[:], fp_p[:])
                zr = smk.tile([1, P], dt.float32, tag="zr")
                v.tensor_tensor_scan(zr[:], a_row[:], f_row[:], 0.0,
                                     Op.mult, Op.add)
                zs = smk.tile([1, P], dt.float32, tag="zs")
                v.memset(zs[:], 0.0)
                v.tensor_copy(zs[0:1, 1:P], zr[0:1, 0:P - 1])
                v.tensor_tensor(out=zs[:], in0=zs[:], in1=startmask, op=Op.mult)
                vip = ps.tile([P, 1], dt.float32, tag="vip")
                te.transpose(vip[:], zs[:], ones11)
                vic = smk.tile([P, 1], dt.float32, tag="vic")
                v.tensor_copy(vic[:], vip[:])
                return vic, zs

            def row_broadcast(pair_row):
                pr = ps.tile([2, 1], dt.float32, tag="pr")
                te.transpose(pr[:], pair_row, ones11)
                prs = smk.tile([2, 1], dt.float32, tag="prs")
                v.tensor_copy(prs[:], pr[:])
                cb = ps.tile([P, 1], dt.float32, tag="cb")
                te.matmul(cb[:], mt, prs[:])
                out = smk.tile([P, 1], dt.float32, tag="bc")
                v.tensor_copy(out[:], cb[:])
                return out

            # ---------- env warm start (single chunk-local pass) ----------
            # xh = x_L - B into S4 (Act); aW tile into S5 (Act); env scan
            # into F16 (fp16 is plenty for a warm start); v0 = xh - env.
            s.activation(S4[:], S3[:], Act.Identity, bias=col(14), scale=1.0)
            s.activation(S5[:], S3[:], Act.Identity, bias=col(13), scale=0.0)
            v.tensor_tensor_scan(F16[:], S5[:], S4[:], 0.0, Op.mult, Op.max)
            v.tensor_tensor(out=S4[:], in0=S4[:], in1=F16[:], op=Op.subtract)
            # vinit0 = shift(v0 chunk ends), masked at row starts
            vzp = ps.tile([1, P], dt.float32, tag="fp_p")
            te.transpose(vzp[:], S4[:, L - 1:L], ident[:])
            vz_row = smk.tile([1, P], dt.float32, tag="f_row")
            v.tensor_copy(vz_row[:], vzp[:])
            vzs = smk.tile([1, P], dt.float32, tag="d1")
            v.memset(vzs[:], 0.0)
            v.tensor_copy(vzs[0:1, 1:P], vz_row[0:1, 0:P - 1])
            v.tensor_tensor(out=vzs[:], in0=vzs[:], in1=startmask, op=Op.mult)
            vip0 = ps.tile([P, 1], dt.float32, tag="vip")
            te.transpose(vip0[:], vzs[:], ones11)
            vinit0 = smk.tile([P, 1], dt.float32, tag="vic")
            v.tensor_copy(vinit0[:], vip0[:])

            vinit = vinit0
            vinit_prev = None
            zrow_hist = [vzs]
            V_cur = S4
            nsw = len(SCHED)

            for k, step in enumerate(SCHED):
                sd = smk.tile([P, 1], dt.float32, tag="sd")
                lbias = col(8)  # lRL
                Vp = V_cur
                vic_used = vinit
                if step == "E":
                    z0, z1, z2 = zrow_hist[-1], zrow_hist[-2], zrow_hist[-3]
                    d1 = smk.tile([1, P], dt.float32, tag="d1")
                    v.tensor_tensor(out=d1[:], in0=z0[:], in1=z1[:], op=Op.subtract)
                    s.activation(d1[:], d1[:], Act.Abs, bias=0.0, scale=1.0)
                    d2 = smk.tile([1, P], dt.float32, tag="d2")
                    v.tensor_tensor(out=d2[:], in0=z1[:], in1=z2[:], op=Op.subtract)
                    s.activation(d2[:], d2[:], Act.Abs, bias=0.0, scale=1.0)
                    rs = smk.tile([1, 2], dt.float32, tag="rs")
                    rs2 = smk.tile([1, 2], dt.float32, tag="rs2")
                    half = P // 2
                    v.tensor_reduce(rs[0:1, 0:1], d1[0:1, 0:half], mybir.AxisListType.X, Op.add)
                    v.tensor_reduce(rs[0:1, 1:2], d1[0:1, half:P], mybir.AxisListType.X, Op.add)
                    v.tensor_reduce(rs2[0:1, 0:1], d2[0:1, 0:half], mybir.AxisListType.X, Op.add)
                    v.tensor_reduce(rs2[0:1, 1:2], d2[0:1, half:P], mybir.AxisListType.X, Op.add)
                    v.tensor_scalar(out=rs2[:], in0=rs2[:], scalar1=1e-30,
                                    scalar2=None, op0=Op.add)
                    rho = smk.tile([1, 2], dt.float32, tag="rho")
                    v.reciprocal(rs2[:], rs2[:])
                    v.tensor_tensor(out=rho[:], in0=rs[:], in1=rs2[:], op=Op.mult)
                    v.tensor_scalar(out=rho[:], in0=rho[:], scalar1=0.95,
                                    scalar2=None, op0=Op.min)
                    om = smk.tile([1, 2], dt.float32, tag="om")
                    v.tensor_scalar(out=om[:], in0=rho[:], scalar1=-1.0,
                                    scalar2=1.0, op0=Op.mult, op1=Op.add)
                    v.reciprocal(om[:], om[:])
                    gam = smk.tile([1, 2], dt.float32, tag="gam")
                    v.tensor_tensor(out=gam[:], in0=rho[:], in1=om[:], op=Op.mult)
                    gcol = row_broadcast(gam[:])
                    gp1 = smk.tile([P, 1], dt.float32, tag="gp1")
                    v.tensor_scalar(out=gp1[:], in0=gcol[:], scalar1=1.0,
                                    scalar2=None, op0=Op.add)
                    # Ve = (1+g)*Vp - g*Vm1, with Vm1 the fp16 shadow of the
                    # sweep-before-previous V.  G = g*Vm1 in-place (fp16, 4x).
                    v.tensor_scalar(out=F16[:], in0=F16[:], scalar1=gcol[:],
                                    scalar2=None, op0=Op.mult)
                    v.scalar_tensor_tensor(out=S4[:], in0=Vp[:], scalar=gp1[:],
                                           in1=F16[:], op0=Op.mult,
                                           op1=Op.subtract)
                    Vp = S4
                    dv = smk.tile([P, 1], dt.float32, tag="dv")
                    v.tensor_tensor(out=dv[:], in0=vinit[:], in1=vinit_prev[:],
                                    op=Op.subtract)
                    vice = smk.tile([P, 1], dt.float32, tag="vice")
                    v.scalar_tensor_tensor(out=vice[:], in0=dv[:], scalar=gcol[:],
                                           in1=vinit[:], op0=Op.mult, op1=Op.add)
                    vic_used = vice
                # blocked d with per-block partial sums (stt bypass/is_gt)
                sdb = smk.tile([P, NB], dt.float32, tag="sdb")
                v.tensor_tensor(out=S5[:, 0:1], in0=vic_used[:], in1=S2[:, 0:1],
                                op=Op.is_gt)
                for b in range(NB):
                    lo = b * LB
                    hi = (b + 1) * LB
                    l2 = max(lo, 1)
                    v.scalar_tensor_tensor(out=S5[:, l2:hi],
                                           in0=Vp[:, l2 - 1:hi - 1],
                                           scalar=1.0, in1=S2[:, l2:hi],
                                           op0=Op.bypass, op1=Op.is_gt,
                                           accum_out=sdb[:, b:b + 1])
                if k == nsw - 1:
                    # decisions consumed D; refill S2 with x for the post
                    for b in range(NB):
                        sl = slice(b * LB, (b + 1) * LB)
                        nc.sync.dma_start(out=S2[:, sl], in_=x_in[:, sl])
                v.tensor_reduce(sd[:], sdb[:], mybir.AxisListType.X, Op.add)
                # fold col-0 decision into the logA bias
                tl = smk.tile([P, 1], dt.float32, tag="tl")
                v.tensor_scalar(out=tl[:], in0=S5[:, 0:1], scalar1=col(9),
                                scalar2=col(8), op0=Op.mult, op1=Op.add)
                lbias = tl[:]
                vinit_used = vic_used
                A_c, t1 = boundary_A(vinit_used[:], sd[:], lbias)
                # a = dA*d + aR with row-start fix a[.,0] via dstar, then
                # chained block scans.  a lives in S5 in-place.
                v.tensor_copy(S5[0:1, 0:1], cst[0:1, 12:13])
                v.tensor_copy(S5[64:65, 0:1], cst[64:65, 12:13])
                for b in range(NB):
                    lo = b * LB
                    hi = (b + 1) * LB
                    if b == 0:
                        v.tensor_scalar(out=S5[:, lo:hi], in0=S5[:, lo:hi],
                                        scalar1=col(7), scalar2=col(6),
                                        op0=Op.mult, op1=Op.add)
                    else:
                        s.activation(S5[:, lo:hi], S5[:, lo:hi], Act.Identity,
                                     bias=col(6), scale=col(7))
                    init_ap = vinit_used[:] if b == 0 else S4[:, lo - 1:lo]
                    v.tensor_tensor_scan(S4[:, lo:hi], S1[:, lo:hi],
                                         S5[:, lo:hi], init_ap,
                                         Op.add, Op.mult)
                V_cur = S4
                # fp16 shadow of V for the E step: keep V_{nsw-3}
                if k == nsw - 3 and "E" in SCHED:
                    s.activation(F16[:], S4[:], Act.Identity, bias=0.0,
                                 scale=1.0)
                vinit_prev = vinit
                vic, zs = boundary_chain(S4, A_c, t1)
                vinit = vic
                zrow_hist.append(zs)
                if len(zrow_hist) > 3:
                    zrow_hist.pop(0)

            # final exact re-scan (blocked) with POST chained per block
            # S2 = x (re-read during last sweep), S3 = x_L, S5 = a (last
            # sweep's), S1 = ND -> gain, S4 = V.
            for b in range(NB):
                lo = b * LB
                hi = (b + 1) * LB
                sl = slice(lo, hi)
                init_ap = vinit[:] if b == 0 else S4[:, lo - 1:lo]
                v.tensor_tensor_scan(S4[:, sl], S1[:, sl], S5[:, sl], init_ap,
                                     Op.add, Op.mult)
                # -y_L = V - x_L;  clip y_L at -96 via Relu on Act:
                # u = relu(96 - (V - x_L));  gain = exp(-RGAIN*u + col15)
                v.tensor_tensor(out=S5[:, sl], in0=S4[:, sl], in1=S3[:, sl],
                                op=Op.subtract)
                s.activation(S5[:, sl], S5[:, sl], Act.Relu, bias=col(16),
                             scale=-1.0)
                s.activation(S1[:, sl], S5[:, sl], Act.Exp, bias=col(15),
                             scale=-float(RGAIN))
                v.tensor_tensor(out=S3[:, sl], in0=S1[:, sl], in1=S2[:, sl],
                                op=Op.mult)
                nc.sync.dma_start(out=y_out[:, sl], in_=S3[:, sl])

    nc.compile()
    return nc


_CACHE = {}
PROFILE = False
LAST_EXEC_NS = None
LAST_RESULTS = None


def _get_program(L):
    if L not in _CACHE:
        _CACHE[L] = build_program(L)
    return _CACHE[L]


def make_core_inputs(x, params, L):
    """Full x [B,N], params [B,6] -> list of per-core input dicts."""
    B, N = x.shape
    n_cores = B // ROWS_PER_CORE
    rows_per_core = ROWS_PER_CORE
    maxabs = np.abs(x).max(axis=1)
    csts, lR = host_consts(params, maxabs)
    csts[:, 8] = (lR.astype(np.float64) * L).astype(f32)
    ident = np.eye(P, dtype=f32)
    in_maps = []
    for c in range(n_cores):
        rows = slice(c * rows_per_core, (c + 1) * rows_per_core)
        xs = np.ascontiguousarray(x[rows]).reshape(P, L)
        cc = np.repeat(csts[rows], P // rows_per_core, axis=0)
        aux = np.zeros((5, P), f32)
        aux[0, :] = 1.0
        aux[0, 0] = 0.0
        aux[0, 64] = 0.0
        aux[1, 0:64] = 1.0   # Mt row 0 -> partitions 0..63
        aux[2, 64:128] = 1.0
        aux[3, 0] = 1.0      # ones11
        in_maps.append(dict(x=xs, cst=np.ascontiguousarray(cc),
                            aux=aux, ident=ident))
    return in_maps


def kernel(x, params):
    x = np.asarray(x)
    params = np.asarray(params, f32)
    B, N = x.shape
    L = (N * ROWS_PER_CORE) // P
    nc = _get_program(L)
    in_maps = make_core_inputs(np.asarray(x, f32), params, L)
    global LAST_EXEC_NS, LAST_RESULTS
    res = run_bass_kernel_spmd(nc, in_maps, list(range(B // ROWS_PER_CORE)),
                               trace=PROFILE)
    LAST_EXEC_NS = res.exec_time_ns
    LAST_RESULTS = res
    outs = [r["y"].reshape(ROWS_PER_CORE, N) for r in res.results]
    return np.concatenate(outs, axis=0).astype(x.dtype, copy=False)


# revision 10
# speedup vs baseline: 1.3452x; 1.0375x over previous
"""DRC layer (dynamic range compressor) Trainium2 Bass kernel.

Per batch row, y = x * 10^(-y_L/20) * 10^(mk/20) where y_L is a branching
one-pole smoother (attack/release) over the static gain curve x_L
computed in dB domain.  Solved by fixed-point iteration: guess y ->
branch decisions -> recurrence is linear -> solve exactly with the
hardware tensor_tensor_scan -> repeat.  In v = x_L - y space the
recurrence is v[n] = a[n]*(v[n-1] - delta[n]), which is one scan op
(state = (negdelta + state) * a).  Cross-chunk carries are solved
exactly each sweep with a tiny transposed scan over per-chunk affine
maps.  Schedule: 3 sweeps + 1 Aitken-extrapolated sweep (gamma from
boundary-delta ratios), then a final re-scan with exact carries.

vs the 376981ns baseline: 2 fewer sweeps (IIIE vs IIIEII), no DRAM
spill of x_L (V_{k-2} shadow kept in fp16 so SBUF fits), env warm start
pass 1 dropped (chunk-local init), -96 clip moved to the Act engine
(Relu), x re-read by DMA during the last sweep into the dead D buffer.

Sharding: data-parallel, 2 batch rows per core x 8 cores.  Each core
packs its 2 rows as [128, 8192] (partitions 0-63 = row 0 in 64 chunks
of 8192 samples, 64-127 = row 1).
"""
import sys
import numpy as np

try:
    from concourse import bass, bacc, mybir
except Exception:  # pragma: no cover
    for p in ("/opt/trn_rl_repo", "/root/.axon_site/_ro/trn_rl_repo"):
        if p not in sys.path:
            sys.path.insert(0, p)
    from concourse import bass, bacc, mybir

from concourse.bass_utils import run_bass_kernel_spmd
from concourse.tile import TileContext

f32 = np.float32
dt = mybir.dt
Op = mybir.AluOpType
Act = mybir.ActivationFunctionType

SR = f32(44100.0)
LOG9 = float(np.log(9.0))
CL = f32(20.0 / np.log(10.0))       # ln -> dB scale
RGAIN = f32(np.log(10.0) / 20.0)    # dB -> ln scale
P = 128                             # partitions
ROWS_PER_CORE = 2
N_CORES = 8
SCHED = "IIIE"                      # I=sweep, E=extrapolated sweep
NCONST = 17


def host_consts(params, maxabs=None):
    """params [R,6] float32 -> per-row constants [R, NCONST] float32.
    Mirrors the reference's float32 arithmetic for the alphas."""
    p = params.astype(f32)
    p = np.where(np.isnan(p), f32(0.0), p)
    p = np.where(p == 0, f32(1e-10), p)
    T = (-p[:, 0] * f32(60.0)).astype(f32)
    ratio = (p[:, 1] * f32(10.0)).astype(f32)
    attack = np.maximum((p[:, 2] / f32(10.0)).astype(f32), f32(1e-4))
    release = np.maximum((p[:, 3] * f32(3.0)).astype(f32), f32(0.005))
    W = (p[:, 4] * f32(24.0)).astype(f32)
    mk = (p[:, 5] * f32(20.0)).astype(f32)
    aA = np.exp((f32(-LOG9) / (SR * attack)).astype(f32)).astype(f32)
    aR = np.exp((f32(-LOG9) / (SR * release)).astype(f32)).astype(f32)
    # derived (host f64 where it only affects our solver internals)
    lA = np.log(aA.astype(np.float64))
    lR = np.log(aR.astype(np.float64))
    c1 = (1.0 - 1.0 / ratio.astype(np.float64)).astype(f32)
    negc2 = (-1.0 / (8.0 * W.astype(np.float64) * ratio.astype(np.float64))).astype(f32)
    CL64 = np.float64(20.0 / np.log(10.0))
    T64 = T.astype(np.float64); W64 = W.astype(np.float64)
    out = np.zeros((p.shape[0], NCONST), f32)
    out[:, 0] = (-c1.astype(np.float64) * T64).astype(f32)   # negc1T
    out[:, 1] = (c1.astype(np.float64) * CL64 / 2).astype(f32)  # c1*CL/2
    out[:, 2] = negc2
    out[:, 3] = (2 * (W64 / 2 + T64) / CL64).astype(f32)     # thr_above (on u2)
    out[:, 4] = (2 * (T64 - W64 / 2) / CL64).astype(f32)     # thr_below (on u2)
    out[:, 5] = (W64 - T64).astype(f32)                      # W - T (square bias)
    out[:, 6] = aR
    out[:, 7] = aA - aR               # dA
    out[:, 8] = 0.0                   # lRL: filled per-L at call site
    out[:, 9] = (lA - lR).astype(f32)  # dal
    out[:, 10] = (mk.astype(np.float64) * np.log(10.0) / 20.0).astype(f32)  # expbias
    out[:, 11] = 1e-16                # eps2: u2 = ln(x^2 + eps2)
    dA = (aA - aR).astype(np.float64)
    dA = np.where(dA == 0, 1e-30, dA)
    out[:, 12] = ((1.0 - aR.astype(np.float64)) / dA).astype(f32)  # dstar
    # env warm start: aW = aR for fast-attack rows else 0; B = x_L lower bound
    out[:, 13] = np.where(aA < f32(0.99), aR, f32(0.0)).astype(f32)
    if maxabs is None:
        maxabs = np.full(p.shape[0], 1e4)
    uhi = 20.0 * np.log10(np.asarray(maxabs, np.float64) + 1e-8) - T64 + 1.0
    knee_min = -2.25 * (W64 ** 2) / (8.0 * W64 * ratio.astype(np.float64))
    c1f = c1.astype(np.float64)
    B = np.minimum(0.0, knee_min)
    B = np.minimum(B, np.where(c1f < 0, c1f * np.maximum(uhi, 0.0), 0.0)) - 1.0
    out[:, 14] = (-B).astype(f32)                            # negB
    # expbias + 96*RGAIN (post: exp(-RGAIN*relu(96-(-y_L)) + col15))
    out[:, 15] = (out[:, 10].astype(np.float64)
                  + 96.0 * np.log(10.0) / 20.0).astype(f32)
    out[:, 16] = 96.0                                        # relu bias
    out_lR = lR.astype(f32)
    return out, out_lR


def build_program(L):
    """Build the SPMD Bass program for chunk length L (8192 for the real
    problem). Returns the compiled Bacc."""
    nc = bacc.Bacc("TRN2", target_bir_lowering=False, debug=False,
                   num_devices=N_CORES)
    x_in = nc.dram_tensor("x", (P, L), dt.float32, kind="ExternalInput")
    cst_in = nc.dram_tensor("cst", (P, NCONST), dt.float32, kind="ExternalInput")
    aux_in = nc.dram_tensor("aux", (5, P), dt.float32, kind="ExternalInput")
    ident_in = nc.dram_tensor("ident", (P, P), dt.float32, kind="ExternalInput")
    y_out = nc.dram_tensor("y", (P, L), dt.float32, kind="ExternalOutput")

    v = nc.vector
    s = nc.scalar
    te = nc.tensor

    NB = 4                      # col blocks for iter/post pipelining
    LB = L // NB
    NBP = 4                     # pre col blocks
    LBP = L // NBP

    with TileContext(nc) as tc:
        with (
            tc.tile_pool(name="big", bufs=1) as big,
            tc.tile_pool(name="sm", bufs=2) as sm,
            tc.tile_pool(name="smk", bufs=4) as smk,
            tc.tile_pool(name="ps", bufs=1, space="PSUM") as ps,
        ):
            # ---- persistent small tiles
            cst = sm.tile([P, NCONST], dt.float32, tag="cst")
            nc.sync.dma_start(out=cst[:], in_=cst_in[:])
            maskt = sm.tile([1, P], dt.float32, tag="maskt")
            nc.sync.dma_start(out=maskt[:], in_=aux_in[0:1, :])
            mtt = sm.tile([2, P], dt.float32, tag="mtt")
            nc.sync.dma_start(out=mtt[:], in_=aux_in[1:3, :])
            onest = sm.tile([1, 1], dt.float32, tag="onest")
            nc.sync.dma_start(out=onest[:], in_=aux_in[3:4, 0:1])
            ident = sm.tile([P, P], dt.float32, tag="ident")
            nc.sync.dma_start(out=ident[:], in_=ident_in[:])
            startmask = maskt[0:1, :]    # [1,128]: 0 at chunk 0 and 64
            mt = mtt[0:2, :]             # [2,128] row-block indicator
            ones11 = onest[0:1, 0:1]     # [1,1] = 1.0

            def col(i):
                return cst[:, i:i + 1]

            # ---- big slots (32KB/partition each; F16 16KB, MB 16KB)
            S1 = big.tile([P, L], dt.float32, tag="S1")  # sq scratch -> x
            S2 = big.tile([P, L], dt.float32, tag="S2")  # x -> u2 -> D
            S3 = big.tile([P, L], dt.float32, tag="S3")  # x_L -> gain
            S4 = big.tile([P, L], dt.float32, tag="S4")  # stage -> xh -> w
            S5 = big.tile([P, L], dt.float32, tag="S5")  # lin/mask -> d/a -> y
            F16 = big.tile([P, L], dt.float16, tag="F16")  # env, then Vm1 shadow
            MB = big.tile([P, LBP], dt.int32, tag="MB")  # above-mask

            # ====== PRE (w-space): x -> x_L, D; env chained per block ======
            # u2 = ln(x^2+eps2) = 2*ln|x|; all u-space consts pre-doubled.
            for b in range(NBP):
                sl = slice(b * LBP, (b + 1) * LBP)
                mb = MB[:, 0:LBP]
                nc.sync.dma_start(out=S4[:, sl], in_=x_in[:, sl])
                s.activation(S1[:, sl], S4[:, sl], Act.Square, bias=0.0,
                             scale=1.0)
                s.activation(S4[:, sl], S1[:, sl], Act.Ln, bias=col(11),
                             scale=1.0)
                s.activation(S1[:, sl], S4[:, sl], Act.Square, bias=col(5),
                             scale=float(CL / 2))
                s.activation(S3[:, sl], S1[:, sl], Act.Identity, bias=0.0,
                             scale=col(2))
                s.activation(S5[:, sl], S4[:, sl], Act.Identity, bias=col(0),
                             scale=col(1))
                v.tensor_scalar(out=mb, in0=S4[:, sl], scalar1=col(3),
                                scalar2=None, op0=Op.is_gt)
                v.copy_predicated(S3[:, sl], mb, S5[:, sl])
                v.tensor_scalar(out=S5[:, sl], in0=S4[:, sl], scalar1=col(4),
                                scalar2=None, op0=Op.is_ge)
                v.tensor_tensor(out=S3[:, sl], in0=S3[:, sl], in1=S5[:, sl],
                                op=Op.mult)
                # S3 = x_L block.  D = x_L[n-1] - x_L[n] into S2
                lo = b * LBP
                hi = (b + 1) * LBP
                v.tensor_tensor(out=S2[:, max(lo, 1):hi],
                                in0=S3[:, max(lo, 1) - 1:hi - 1],
                                in1=S3[:, max(lo, 1):hi], op=Op.subtract)
                # env warm start, chained per block: xh -> S4, aW -> S5,
                # decaying max into F16 (fp16 is plenty for a warm start)
                s.activation(S4[:, sl], S3[:, sl], Act.Identity, bias=col(14),
                             scale=1.0)
                s.activation(S5[:, sl], S3[:, sl], Act.Identity, bias=col(13),
                             scale=0.0)
                env_init = 0.0 if b == 0 else F16[:, lo - 1:lo]
                v.tensor_tensor_scan(F16[:, sl], S5[:, sl], S4[:, sl],
                                     env_init, Op.mult, Op.max)
            # cross-chunk delta col 0: prevlast[p] = x_L[p-1, L-1], rows reset 0
            pl = smk.tile([P, 1], dt.float32, tag="pl")
            v.memset(pl[:], 0.0)
            nc.sync.dma_start(out=pl[1:P, :], in_=S3[0:P - 1, L - 1:L])
            v.memset(pl[64:65, :], 0.0)
            v.memset(pl[0:1, :], 0.0)
            v.tensor_tensor(out=S2[:, 0:1], in0=pl[:], in1=S3[:, 0:1],
                            op=Op.subtract)
            # x prefetch for the post phase: S1 is free from here on
            for b in range(NB):
                sl = slice(b * LB, (b + 1) * LB)
                nc.sync.dma_start(out=S1[:, sl], in_=x_in[:, sl])

            # ================= iteration machinery =================
            def boundary_A(vinit_used, sd, bias_ap):
                """A-column and A*winit: only needs sum(d); overlaps scans."""
                logA = smk.tile([P, 1], dt.float32, tag="logA")
                v.scalar_tensor_tensor(out=logA[:], in0=sd, scalar=col(9),
                                       in1=bias_ap, op0=Op.mult, op1=Op.add)
                A_c = smk.tile([P, 1], dt.float32, tag="A_c")
                s.activation(A_c[:], logA[:], Act.Exp, bias=0.0, scale=1.0)
                t1 = smk.tile([P, 1], dt.float32, tag="t1")
                v.tensor_tensor(out=t1[:], in0=A_c[:], in1=vinit_used,
                                op=Op.mult)
                return A_c, t1

            def boundary_chain(V_t, A_c, t1):
                f_c = smk.tile([P, 1], dt.float32, tag="f_c")
                v.tensor_tensor(out=f_c[:], in0=V_t[:, L - 1:L], in1=t1[:],
                                op=Op.subtract)
                ap_p = ps.tile([1, P], dt.float32, tag="ap_p")
                te.transpose(ap_p[:], A_c[:], ident[:])
                a_row = smk.tile([1, P], dt.float32, tag="a_row")
                v.tensor_tensor(out=a_row[:], in0=ap_p[:], in1=startmask,
                                op=Op.mult)
                fp_p = ps.tile([1, P], dt.float32, tag="fp_p")
                te.transpose(fp_p[:], f_c[:], ident[:])
                f_row = smk.tile([1, P], dt.float32, tag="f_row")
                v.tensor_copy(f_row[:], fp_p[:])
                zr = smk.tile([1, P], dt.float32, tag="zr")
                v.tensor_tensor_scan(zr[:], a_row[:], f_row[:], 0.0,
                                     Op.mult, Op.add)
                zs = smk.tile([1, P], dt.float32, tag="zs")
                v.memset(zs[:], 0.0)
                v.tensor_copy(zs[0:1, 1:P], zr[0:1, 0:P - 1])
                v.tensor_tensor(out=zs[:], in0=zs[:], in1=startmask, op=Op.mult)
                vip = ps.tile([P, 1], dt.float32, tag="vip")
                te.transpose(vip[:], zs[:], ones11)
                vic = smk.tile([P, 1], dt.float32, tag="vic")
                v.tensor_copy(vic[:], vip[:])
                return vic, zs

            def row_broadcast(pair_row):
                pr = ps.tile([2, 1], dt.float32, tag="pr")
                te.transpose(pr[:], pair_row, ones11)
                prs = smk.tile([2, 1], dt.float32, tag="prs")
                v.tensor_copy(prs[:], pr[:])
                cb = ps.tile([P, 1], dt.float32, tag="cb")
                te.matmul(cb[:], mt, prs[:])
                out = smk.tile([P, 1], dt.float32, tag="bc")
                v.tensor_copy(out[:], cb[:])
                return out

            # w0 = env - xh;  winit0 = shift(w0 chunk ends) masked at rows
            v.tensor_tensor(out=S4[:], in0=F16[:], in1=S4[:], op=Op.subtract)
            vzp = ps.tile([1, P], dt.float32, tag="fp_p")
            te.transpose(vzp[:], S4[:, L - 1:L], ident[:])
            vz_row = smk.tile([1, P], dt.float32, tag="f_row")
            v.tensor_copy(vz_row[:], vzp[:])
            vzs = smk.tile([1, P], dt.float32, tag="d1")
            v.memset(vzs[:], 0.0)
            v.tensor_copy(vzs[0:1, 1:P], vz_row[0:1, 0:P - 1])
            v.tensor_tensor(out=vzs[:], in0=vzs[:], in1=startmask, op=Op.mult)
            vip0 = ps.tile([P, 1], dt.float32, tag="vip")
            te.transpose(vip0[:], vzs[:], ones11)
            vinit0 = smk.tile([P, 1], dt.float32, tag="vic")
            v.tensor_copy(vinit0[:], vip0[:])

            vinit = vinit0
            vinit_prev = None
            zrow_hist = [vzs]
            pending = None          # (A_c, t1) of the un-resolved chain
            nsw = len(SCHED)

            def resolve_chain():
                nonlocal vinit, vinit_prev, pending
                vic, zs = boundary_chain(S4, *pending)
                pending = None
                vinit_prev = vinit
                vinit = vic
                zrow_hist.append(zs)
                if len(zrow_hist) > 3:
                    zrow_hist.pop(0)

            for k, step in enumerate(SCHED):
                sd = smk.tile([P, 1], dt.float32, tag="sd")
                sdb = smk.tile([P, NB], dt.float32, tag="sdb")
                Vp = S4
                if step == "E":
                    # gamma needs z_{k-1}: resolve the chain first
                    if pending is not None:
                        resolve_chain()
                    z0, z1, z2 = zrow_hist[-1], zrow_hist[-2], zrow_hist[-3]
                    d1 = smk.tile([1, P], dt.float32, tag="d1")
                    v.tensor_tensor(out=d1[:], in0=z0[:], in1=z1[:], op=Op.subtract)
                    s.activation(d1[:], d1[:], Act.Abs, bias=0.0, scale=1.0)
                    d2 = smk.tile([1, P], dt.float32, tag="d2")
                    v.tensor_tensor(out=d2[:], in0=z1[:], in1=z2[:], op=Op.subtract)
                    s.activation(d2[:], d2[:], Act.Abs, bias=0.0, scale=1.0)
                    rs = smk.tile([1, 2], dt.float32, tag="rs")
                    rs2 = smk.tile([1, 2], dt.float32, tag="rs2")
                    half = P // 2
                    v.tensor_reduce(rs[0:1, 0:1], d1[0:1, 0:half], mybir.AxisListType.X, Op.add)
                    v.tensor_reduce(rs[0:1, 1:2], d1[0:1, half:P], mybir.AxisListType.X, Op.add)
                    v.tensor_reduce(rs2[0:1, 0:1], d2[0:1, 0:half], mybir.AxisListType.X, Op.add)
                    v.tensor_reduce(rs2[0:1, 1:2], d2[0:1, half:P], mybir.AxisListType.X, Op.add)
                    v.tensor_scalar(out=rs2[:], in0=rs2[:], scalar1=1e-30,
                                    scalar2=None, op0=Op.add)
                    rho = smk.tile([1, 2], dt.float32, tag="rho")
                    v.reciprocal(rs2[:], rs2[:])
                    v.tensor_tensor(out=rho[:], in0=rs[:], in1=rs2[:], op=Op.mult)
                    v.tensor_scalar(out=rho[:], in0=rho[:], scalar1=0.95,
                                    scalar2=None, op0=Op.min)
                    om = smk.tile([1, 2], dt.float32, tag="om")
                    v.tensor_scalar(out=om[:], in0=rho[:], scalar1=-1.0,
                                    scalar2=1.0, op0=Op.mult, op1=Op.add)
                    v.reciprocal(om[:], om[:])
                    gam = smk.tile([1, 2], dt.float32, tag="gam")
                    v.tensor_tensor(out=gam[:], in0=rho[:], in1=om[:], op=Op.mult)
                    gcol = row_broadcast(gam[:])
                    gp1 = smk.tile([P, 1], dt.float32, tag="gp1")
                    v.tensor_scalar(out=gp1[:], in0=gcol[:], scalar1=1.0,
                                    scalar2=None, op0=Op.add)
                    # Ve = (1+g)*Vp - g*Vm1 (fp16 shadow); G=g*Vm1 in place
                    v.tensor_scalar(out=F16[:], in0=F16[:], scalar1=gcol[:],
                                    scalar2=None, op0=Op.mult)
                    v.scalar_tensor_tensor(out=S4[:], in0=Vp[:], scalar=gp1[:],
                                           in1=F16[:], op0=Op.mult,
                                           op1=Op.subtract)
                    dv = smk.tile([P, 1], dt.float32, tag="dv")
                    v.tensor_tensor(out=dv[:], in0=vinit[:], in1=vinit_prev[:],
                                    op=Op.subtract)
                    vice = smk.tile([P, 1], dt.float32, tag="vice")
                    v.scalar_tensor_tensor(out=vice[:], in0=dv[:], scalar=gcol[:],
                                           in1=vinit[:], op0=Op.mult, op1=Op.add)
                    vinit = vice
                # decision blocks: d = (-w[n-1] > D[n]); no vinit needed
                for b in range(NB):
                    lo = b * LB
                    hi = (b + 1) * LB
                    l2 = max(lo, 1)
                    v.scalar_tensor_tensor(out=S5[:, l2:hi],
                                           in0=Vp[:, l2 - 1:hi - 1],
                                           scalar=-1.0, in1=S2[:, l2:hi],
                                           op0=Op.mult, op1=Op.is_gt,
                                           accum_out=sdb[:, b:b + 1])
                # resolve the previous sweep's boundary chain (overlaps the
                # decision blocks above in the DVE queue)
                if pending is not None:
                    resolve_chain()
                # col-0 decision needs winit
                v.scalar_tensor_tensor(out=S5[:, 0:1], in0=vinit[:],
                                       scalar=-1.0, in1=S2[:, 0:1],
                                       op0=Op.mult, op1=Op.is_gt)
                v.tensor_reduce(sd[:], sdb[:], mybir.AxisListType.X, Op.add)
                # fold col-0 decision into the logA bias
                tl = smk.tile([P, 1], dt.float32, tag="tl")
                v.tensor_scalar(out=tl[:], in0=S5[:, 0:1], scalar1=col(9),
                                scalar2=col(8), op0=Op.mult, op1=Op.add)
                A_c, t1 = boundary_A(vinit[:], sd[:], tl[:])
                # a = dA*d + aR with row-start fix via dstar; in-place in S5
                v.tensor_copy(S5[0:1, 0:1], cst[0:1, 12:13])
                v.tensor_copy(S5[64:65, 0:1], cst[64:65, 12:13])
                for b in range(NB):
                    lo = b * LB
                    hi = (b + 1) * LB
                    if b == 0:
                        v.tensor_scalar(out=S5[:, lo:hi], in0=S5[:, lo:hi],
                                        scalar1=col(7), scalar2=col(6),
                                        op0=Op.mult, op1=Op.add)
                    else:
                        s.activation(S5[:, lo:hi], S5[:, lo:hi], Act.Identity,
                                     bias=col(6), scale=col(7))
                    init_ap = vinit[:] if b == 0 else S4[:, lo - 1:lo]
                    v.tensor_tensor_scan(S4[:, lo:hi], S2[:, lo:hi],
                                         S5[:, lo:hi], init_ap,
                                         Op.add, Op.mult)
                # fp16 shadow of w for the E step: keep V_{nsw-3}
                if k == nsw - 3 and "E" in SCHED:
                    s.activation(F16[:], S4[:], Act.Identity, bias=0.0,
                                 scale=1.0)
                pending = (A_c, t1)

            resolve_chain()

            # final exact re-scan (blocked) with POST chained per block
            # S1 = x (prefetched), S2 = D, S3 = x_L -> gain, S5 = a -> y
            NBF = 8
            LBF = L // NBF
            for b in range(NBF):
                lo = b * LBF
                hi = (b + 1) * LBF
                sl = slice(lo, hi)
                init_ap = vinit[:] if b == 0 else S4[:, lo - 1:lo]
                v.tensor_tensor_scan(S4[:, sl], S2[:, sl], S5[:, sl], init_ap,
                                     Op.add, Op.mult)
                # y_L = w + x_L; clip via Relu; gain = exp(-RGAIN*u + col15)
                v.tensor_tensor(out=S5[:, sl], in0=S4[:, sl], in1=S3[:, sl],
                                op=Op.add)
                s.activation(S5[:, sl], S5[:, sl], Act.Relu, bias=col(16),
                             scale=1.0)
                s.activation(S3[:, sl], S5[:, sl], Act.Exp, bias=col(15),
                             scale=-float(RGAIN))
                v.tensor_tensor(out=S5[:, sl], in0=S3[:, sl], in1=S1[:, sl],
                                op=Op.mult)
                nc.sync.dma_start(out=y_out[:, sl], in_=S5[:, sl])

    nc.compile()
    return nc


_CACHE = {}
PROFILE = False
LAST_EXEC_NS = None
LAST_RESULTS = None


def _get_program(L):
    if L not in _CACHE:
        _CACHE[L] = build_program(L)
    return _CACHE[L]


def make_core_inputs(x, params, L):
    """Full x [B,N], params [B,6] -> list of per-core input dicts."""
    B, N = x.shape
    n_cores = B // ROWS_PER_CORE
    rows_per_core = ROWS_PER_CORE
    maxabs = np.abs(x).max(axis=1)
    csts, lR = host_consts(params, maxabs)
    csts[:, 8] = (lR.astype(np.float64) * L).astype(f32)
    ident = np.eye(P, dtype=f32)
    in_maps = []
    for c in range(n_cores):
        rows = slice(c * rows_per_core, (c + 1) * rows_per_core)
        xs = np.ascontiguousarray(x[rows]).reshape(P, L)
        cc = np.repeat(csts[rows], P // rows_per_core, axis=0)
        aux = np.zeros((5, P), f32)
        aux[0, :] = 1.0
        aux[0, 0] = 0.0
        aux[0, 64] = 0.0
        aux[1, 0:64] = 1.0   # Mt row 0 -> partitions 0..63
        aux[2, 64:128] = 1.0
        aux[3, 0] = 1.0      # ones11
        in_maps.append(dict(x=xs, cst=np.ascontiguousarray(cc),
                            aux=aux, ident=ident))
    return in_maps


def kernel(x, params):
    x = np.asarray(x)
    params = np.asarray(params, f32)
    B, N = x.shape
    L = (N * ROWS_PER_CORE) // P
    nc = _get_program(L)
    in_maps = make_core_inputs(np.asarray(x, f32), params, L)
    global LAST_EXEC_NS, LAST_RESULTS
    res = run_bass_kernel_spmd(nc, in_maps, list(range(B // ROWS_PER_CORE)),
                               trace=PROFILE)
    LAST_EXEC_NS = res.exec_time_ns
    LAST_RESULTS = res
    outs = [r["y"].reshape(ROWS_PER_CORE, N) for r in res.results]
    return np.concatenate(outs, axis=0).astype(x.dtype, copy=False)


# revision 11
# speedup vs baseline: 1.3518x; 1.0049x over previous
"""DRC layer (dynamic range compressor) Trainium2 Bass kernel.

Per batch row, y = x * 10^(-y_L/20) * 10^(mk/20) where y_L is a branching
one-pole smoother (attack/release) over the static gain curve x_L
computed in dB domain.  Solved by fixed-point iteration: guess y ->
branch decisions -> recurrence is linear -> solve exactly with the
hardware tensor_tensor_scan -> repeat.  In v = x_L - y space the
recurrence is v[n] = a[n]*(v[n-1] - delta[n]), which is one scan op
(state = (negdelta + state) * a).  Cross-chunk carries are solved
exactly each sweep with a tiny transposed scan over per-chunk affine
maps.  Schedule: 3 sweeps + 1 Aitken-extrapolated sweep (gamma from
boundary-delta ratios), then a final re-scan with exact carries.

vs the 376981ns baseline: 2 fewer sweeps (IIIE vs IIIEII), no DRAM
spill of x_L (V_{k-2} shadow kept in fp16 so SBUF fits), env warm start
pass 1 dropped (chunk-local init), -96 clip moved to the Act engine
(Relu), x re-read by DMA during the last sweep into the dead D buffer.

Sharding: data-parallel, 2 batch rows per core x 8 cores.  Each core
packs its 2 rows as [128, 8192] (partitions 0-63 = row 0 in 64 chunks
of 8192 samples, 64-127 = row 1).
"""
import sys
import numpy as np

try:
    from concourse import bass, bacc, mybir
except Exception:  # pragma: no cover
    for p in ("/opt/trn_rl_repo", "/root/.axon_site/_ro/trn_rl_repo"):
        if p not in sys.path:
            sys.path.insert(0, p)
    from concourse import bass, bacc, mybir

from concourse.bass_utils import run_bass_kernel_spmd
from concourse.tile import TileContext

f32 = np.float32
dt = mybir.dt
Op = mybir.AluOpType
Act = mybir.ActivationFunctionType

SR = f32(44100.0)
LOG9 = float(np.log(9.0))
CL = f32(20.0 / np.log(10.0))       # ln -> dB scale
RGAIN = f32(np.log(10.0) / 20.0)    # dB -> ln scale
P = 128                             # partitions
ROWS_PER_CORE = 2
N_CORES = 8
SCHED = "IIIE"                      # I=sweep, E=extrapolated sweep
NCONST = 17


def host_consts(params, maxabs=None):
    """params [R,6] float32 -> per-row constants [R, NCONST] float32.
    Mirrors the reference's float32 arithmetic for the alphas."""
    p = params.astype(f32)
    p = np.where(np.isnan(p), f32(0.0), p)
    p = np.where(p == 0, f32(1e-10), p)
    T = (-p[:, 0] * f32(60.0)).astype(f32)
    ratio = (p[:, 1] * f32(10.0)).astype(f32)
    attack = np.maximum((p[:, 2] / f32(10.0)).astype(f32), f32(1e-4))
    release = np.maximum((p[:, 3] * f32(3.0)).astype(f32), f32(0.005))
    W = (p[:, 4] * f32(24.0)).astype(f32)
    mk = (p[:, 5] * f32(20.0)).astype(f32)
    aA = np.exp((f32(-LOG9) / (SR * attack)).astype(f32)).astype(f32)
    aR = np.exp((f32(-LOG9) / (SR * release)).astype(f32)).astype(f32)
    # derived (host f64 where it only affects our solver internals)
    lA = np.log(aA.astype(np.float64))
    lR = np.log(aR.astype(np.float64))
    c1 = (1.0 - 1.0 / ratio.astype(np.float64)).astype(f32)
    negc2 = (-1.0 / (8.0 * W.astype(np.float64) * ratio.astype(np.float64))).astype(f32)
    CL64 = np.float64(20.0 / np.log(10.0))
    T64 = T.astype(np.float64); W64 = W.astype(np.float64)
    out = np.zeros((p.shape[0], NCONST), f32)
    out[:, 0] = (-c1.astype(np.float64) * T64).astype(f32)   # negc1T
    out[:, 1] = (c1.astype(np.float64) * CL64 / 2).astype(f32)  # c1*CL/2
    out[:, 2] = negc2
    out[:, 3] = (2 * (W64 / 2 + T64) / CL64).astype(f32)     # thr_above (on u2)
    out[:, 4] = (2 * (T64 - W64 / 2) / CL64).astype(f32)     # thr_below (on u2)
    out[:, 5] = (W64 - T64).astype(f32)                      # W - T (square bias)
    out[:, 6] = aR
    out[:, 7] = aA - aR               # dA
    out[:, 8] = 0.0                   # lRL: filled per-L at call site
    out[:, 9] = (lA - lR).astype(f32)  # dal
    out[:, 10] = (mk.astype(np.float64) * np.log(10.0) / 20.0).astype(f32)  # expbias
    out[:, 11] = 1e-16                # eps2: u2 = ln(x^2 + eps2)
    dA = (aA - aR).astype(np.float64)
    dA = np.where(dA == 0, 1e-30, dA)
    out[:, 12] = ((1.0 - aR.astype(np.float64)) / dA).astype(f32)  # dstar
    # env warm start: aW = aR for fast-attack rows else 0; B = x_L lower bound
    out[:, 13] = np.where(aA < f32(0.99), aR, f32(0.0)).astype(f32)
    if maxabs is None:
        maxabs = np.full(p.shape[0], 1e4)
    uhi = 20.0 * np.log10(np.asarray(maxabs, np.float64) + 1e-8) - T64 + 1.0
    knee_min = -2.25 * (W64 ** 2) / (8.0 * W64 * ratio.astype(np.float64))
    c1f = c1.astype(np.float64)
    B = np.minimum(0.0, knee_min)
    B = np.minimum(B, np.where(c1f < 0, c1f * np.maximum(uhi, 0.0), 0.0)) - 1.0
    out[:, 14] = (-B).astype(f32)                            # negB
    # expbias + 96*RGAIN (post: exp(-RGAIN*relu(96-(-y_L)) + col15))
    out[:, 15] = (out[:, 10].astype(np.float64)
                  + 96.0 * np.log(10.0) / 20.0).astype(f32)
    out[:, 16] = 96.0                                        # relu bias
    out_lR = lR.astype(f32)
    return out, out_lR


def build_program(L):
    """Build the SPMD Bass program for chunk length L (8192 for the real
    problem). Returns the compiled Bacc."""
    nc = bacc.Bacc("TRN2", target_bir_lowering=False, debug=False,
                   num_devices=N_CORES)
    x_in = nc.dram_tensor("x", (P, L), dt.float32, kind="ExternalInput")
    cst_in = nc.dram_tensor("cst", (P, NCONST), dt.float32, kind="ExternalInput")
    aux_in = nc.dram_tensor("aux", (5, P), dt.float32, kind="ExternalInput")
    ident_in = nc.dram_tensor("ident", (P, P), dt.float32, kind="ExternalInput")
    y_out = nc.dram_tensor("y", (P, L), dt.float32, kind="ExternalOutput")

    v = nc.vector
    s = nc.scalar
    te = nc.tensor

    NB = 4                      # col blocks for iter/post pipelining
    LB = L // NB
    NBP = 4                     # pre col blocks
    LBP = L // NBP

    with TileContext(nc) as tc:
        with (
            tc.tile_pool(name="big", bufs=1) as big,
            tc.tile_pool(name="sm", bufs=2) as sm,
            tc.tile_pool(name="smk", bufs=4) as smk,
            tc.tile_pool(name="ps", bufs=1, space="PSUM") as ps,
        ):
            # ---- persistent small tiles
            cst = sm.tile([P, NCONST], dt.float32, tag="cst")
            nc.sync.dma_start(out=cst[:], in_=cst_in[:])
            maskt = sm.tile([1, P], dt.float32, tag="maskt")
            nc.sync.dma_start(out=maskt[:], in_=aux_in[0:1, :])
            mtt = sm.tile([2, P], dt.float32, tag="mtt")
            nc.sync.dma_start(out=mtt[:], in_=aux_in[1:3, :])
            onest = sm.tile([1, 1], dt.float32, tag="onest")
            nc.sync.dma_start(out=onest[:], in_=aux_in[3:4, 0:1])
            ident = sm.tile([P, P], dt.float32, tag="ident")
            nc.sync.dma_start(out=ident[:], in_=ident_in[:])
            startmask = maskt[0:1, :]    # [1,128]: 0 at chunk 0 and 64
            mt = mtt[0:2, :]             # [2,128] row-block indicator
            ones11 = onest[0:1, 0:1]     # [1,1] = 1.0

            def col(i):
                return cst[:, i:i + 1]

            # ---- big slots (32KB/partition each; F16 16KB, MB 16KB)
            S1 = big.tile([P, L], dt.float32, tag="S1")  # sq scratch -> x
            S2 = big.tile([P, L], dt.float32, tag="S2")  # x -> u2 -> D
            S3 = big.tile([P, L], dt.float32, tag="S3")  # x_L -> gain
            S4 = big.tile([P, L], dt.float32, tag="S4")  # stage -> xh -> w
            S5 = big.tile([P, L], dt.float32, tag="S5")  # lin/mask -> d/a -> y
            F16 = big.tile([P, L], dt.float16, tag="F16")  # env, then Vm1 shadow
            MB = big.tile([P, LBP], dt.int32, tag="MB")  # above-mask

            # ====== PRE (w-space): x -> x_L, D; env chained per block ======
            # u2 = ln(x^2+eps2) = 2*ln|x|; all u-space consts pre-doubled.
            # smaller lead blocks shorten the serial Act-chain ramp.
            pre_edges = [0, L // 8, L // 4, L // 2, 3 * L // 4, L]
            for b in range(len(pre_edges) - 1):
                lo = pre_edges[b]
                hi = pre_edges[b + 1]
                sl = slice(lo, hi)
                mb = MB[:, 0:hi - lo]
                nc.sync.dma_start(out=S4[:, sl], in_=x_in[:, sl])
                s.activation(S1[:, sl], S4[:, sl], Act.Square, bias=0.0,
                             scale=1.0)
                s.activation(S4[:, sl], S1[:, sl], Act.Ln, bias=col(11),
                             scale=1.0)
                s.activation(S1[:, sl], S4[:, sl], Act.Square, bias=col(5),
                             scale=float(CL / 2))
                s.activation(S3[:, sl], S1[:, sl], Act.Identity, bias=0.0,
                             scale=col(2))
                s.activation(S5[:, sl], S4[:, sl], Act.Identity, bias=col(0),
                             scale=col(1))
                v.tensor_scalar(out=mb, in0=S4[:, sl], scalar1=col(3),
                                scalar2=None, op0=Op.is_gt)
                v.copy_predicated(S3[:, sl], mb, S5[:, sl])
                v.tensor_scalar(out=S5[:, sl], in0=S4[:, sl], scalar1=col(4),
                                scalar2=None, op0=Op.is_ge)
                v.tensor_tensor(out=S3[:, sl], in0=S3[:, sl], in1=S5[:, sl],
                                op=Op.mult)
                # S3 = x_L block.  D = x_L[n-1] - x_L[n] into S2
                v.tensor_tensor(out=S2[:, max(lo, 1):hi],
                                in0=S3[:, max(lo, 1) - 1:hi - 1],
                                in1=S3[:, max(lo, 1):hi], op=Op.subtract)
                # env warm start, chained per block: xh -> S4, aW -> S5,
                # decaying max into F16 (fp16 is plenty for a warm start)
                s.activation(S4[:, sl], S3[:, sl], Act.Identity, bias=col(14),
                             scale=1.0)
                s.activation(S5[:, sl], S3[:, sl], Act.Identity, bias=col(13),
                             scale=0.0)
                env_init = 0.0 if lo == 0 else F16[:, lo - 1:lo]
                v.tensor_tensor_scan(F16[:, sl], S5[:, sl], S4[:, sl],
                                     env_init, Op.mult, Op.max)
            # cross-chunk delta col 0: prevlast[p] = x_L[p-1, L-1], rows reset 0
            pl = smk.tile([P, 1], dt.float32, tag="pl")
            v.memset(pl[:], 0.0)
            nc.sync.dma_start(out=pl[1:P, :], in_=S3[0:P - 1, L - 1:L])
            v.memset(pl[64:65, :], 0.0)
            v.memset(pl[0:1, :], 0.0)
            v.tensor_tensor(out=S2[:, 0:1], in0=pl[:], in1=S3[:, 0:1],
                            op=Op.subtract)
            # x prefetch for the post phase: S1 is free from here on
            for b in range(NB):
                sl = slice(b * LB, (b + 1) * LB)
                nc.sync.dma_start(out=S1[:, sl], in_=x_in[:, sl])

            # ================= iteration machinery =================
            def boundary_A(vinit_used, sd, bias_ap):
                """A-column and A*winit: only needs sum(d); overlaps scans."""
                logA = smk.tile([P, 1], dt.float32, tag="logA")
                v.scalar_tensor_tensor(out=logA[:], in0=sd, scalar=col(9),
                                       in1=bias_ap, op0=Op.mult, op1=Op.add)
                A_c = smk.tile([P, 1], dt.float32, tag="A_c")
                s.activation(A_c[:], logA[:], Act.Exp, bias=0.0, scale=1.0)
                t1 = smk.tile([P, 1], dt.float32, tag="t1")
                v.tensor_tensor(out=t1[:], in0=A_c[:], in1=vinit_used,
                                op=Op.mult)
                return A_c, t1

            def boundary_chain(V_t, A_c, t1):
                f_c = smk.tile([P, 1], dt.float32, tag="f_c")
                v.tensor_tensor(out=f_c[:], in0=V_t[:, L - 1:L], in1=t1[:],
                                op=Op.subtract)
                ap_p = ps.tile([1, P], dt.float32, tag="ap_p")
                te.transpose(ap_p[:], A_c[:], ident[:])
                a_row = smk.tile([1, P], dt.float32, tag="a_row")
                v.tensor_tensor(out=a_row[:], in0=ap_p[:], in1=startmask,
                                op=Op.mult)
                fp_p = ps.tile([1, P], dt.float32, tag="fp_p")
                te.transpose(fp_p[:], f_c[:], ident[:])
                f_row = smk.tile([1, P], dt.float32, tag="f_row")
                v.tensor_copy(f_row[:], fp_p[:])
                zr = smk.tile([1, P], dt.float32, tag="zr")
                v.tensor_tensor_scan(zr[:], a_row[:], f_row[:], 0.0,
                                     Op.mult, Op.add)
                zs = smk.tile([1, P], dt.float32, tag="zs")
                v.memset(zs[:], 0.0)
                v.tensor_copy(zs[0:1, 1:P], zr[0:1, 0:P - 1])
                v.tensor_tensor(out=zs[:], in0=zs[:], in1=startmask, op=Op.mult)
                vip = ps.tile([P, 1], dt.float32, tag="vip")
                te.transpose(vip[:], zs[:], ones11)
                vic = smk.tile([P, 1], dt.float32, tag="vic")
                v.tensor_copy(vic[:], vip[:])
                return vic, zs

            def row_broadcast(pair_row):
                pr = ps.tile([2, 1], dt.float32, tag="pr")
                te.transpose(pr[:], pair_row, ones11)
                prs = smk.tile([2, 1], dt.float32, tag="prs")
                v.tensor_copy(prs[:], pr[:])
                cb = ps.tile([P, 1], dt.float32, tag="cb")
                te.matmul(cb[:], mt, prs[:])
                out = smk.tile([P, 1], dt.float32, tag="bc")
                v.tensor_copy(out[:], cb[:])
                return out

            # w0 = env - xh;  winit0 = shift(w0 chunk ends) masked at rows
            v.tensor_tensor(out=S4[:], in0=F16[:], in1=S4[:], op=Op.subtract)
            vzp = ps.tile([1, P], dt.float32, tag="fp_p")
            te.transpose(vzp[:], S4[:, L - 1:L], ident[:])
            vz_row = smk.tile([1, P], dt.float32, tag="f_row")
            v.tensor_copy(vz_row[:], vzp[:])
            vzs = smk.tile([1, P], dt.float32, tag="d1")
            v.memset(vzs[:], 0.0)
            v.tensor_copy(vzs[0:1, 1:P], vz_row[0:1, 0:P - 1])
            v.tensor_tensor(out=vzs[:], in0=vzs[:], in1=startmask, op=Op.mult)
            vip0 = ps.tile([P, 1], dt.float32, tag="vip")
            te.transpose(vip0[:], vzs[:], ones11)
            vinit0 = smk.tile([P, 1], dt.float32, tag="vic")
            v.tensor_copy(vinit0[:], vip0[:])

            vinit = vinit0
            vinit_prev = None
            zrow_hist = [vzs]
            pending = None          # (A_c, t1) of the un-resolved chain
            nsw = len(SCHED)

            def resolve_chain():
                nonlocal vinit, vinit_prev, pending
                vic, zs = boundary_chain(S4, *pending)
                pending = None
                vinit_prev = vinit
                vinit = vic
                zrow_hist.append(zs)
                if len(zrow_hist) > 3:
                    zrow_hist.pop(0)

            for k, step in enumerate(SCHED):
                sd = smk.tile([P, 1], dt.float32, tag="sd")
                sdb = smk.tile([P, NB], dt.float32, tag="sdb")
                Vp = S4
                if step == "E":
                    # gamma needs z_{k-1}: resolve the chain first
                    if pending is not None:
                        resolve_chain()
                    z0, z1, z2 = zrow_hist[-1], zrow_hist[-2], zrow_hist[-3]
                    d1 = smk.tile([1, P], dt.float32, tag="d1")
                    v.tensor_tensor(out=d1[:], in0=z0[:], in1=z1[:], op=Op.subtract)
                    s.activation(d1[:], d1[:], Act.Abs, bias=0.0, scale=1.0)
                    d2 = smk.tile([1, P], dt.float32, tag="d2")
                    v.tensor_tensor(out=d2[:], in0=z1[:], in1=z2[:], op=Op.subtract)
                    s.activation(d2[:], d2[:], Act.Abs, bias=0.0, scale=1.0)
                    rs = smk.tile([1, 2], dt.float32, tag="rs")
                    rs2 = smk.tile([1, 2], dt.float32, tag="rs2")
                    half = P // 2
                    v.tensor_reduce(rs[0:1, 0:1], d1[0:1, 0:half], mybir.AxisListType.X, Op.add)
                    v.tensor_reduce(rs[0:1, 1:2], d1[0:1, half:P], mybir.AxisListType.X, Op.add)
                    v.tensor_reduce(rs2[0:1, 0:1], d2[0:1, 0:half], mybir.AxisListType.X, Op.add)
                    v.tensor_reduce(rs2[0:1, 1:2], d2[0:1, half:P], mybir.AxisListType.X, Op.add)
                    v.tensor_scalar(out=rs2[:], in0=rs2[:], scalar1=1e-30,
                                    scalar2=None, op0=Op.add)
                    rho = smk.tile([1, 2], dt.float32, tag="rho")
                    v.reciprocal(rs2[:], rs2[:])
                    v.tensor_tensor(out=rho[:], in0=rs[:], in1=rs2[:], op=Op.mult)
                    v.tensor_scalar(out=rho[:], in0=rho[:], scalar1=0.95,
                                    scalar2=None, op0=Op.min)
                    om = smk.tile([1, 2], dt.float32, tag="om")
                    v.tensor_scalar(out=om[:], in0=rho[:], scalar1=-1.0,
                                    scalar2=1.0, op0=Op.mult, op1=Op.add)
                    v.reciprocal(om[:], om[:])
                    gam = smk.tile([1, 2], dt.float32, tag="gam")
                    v.tensor_tensor(out=gam[:], in0=rho[:], in1=om[:], op=Op.mult)
                    gcol = row_broadcast(gam[:])
                    gp1 = smk.tile([P, 1], dt.float32, tag="gp1")
                    v.tensor_scalar(out=gp1[:], in0=gcol[:], scalar1=1.0,
                                    scalar2=None, op0=Op.add)
                    # Ve = (1+g)*Vp - g*Vm1 (fp16 shadow); G=g*Vm1 in place
                    v.tensor_scalar(out=F16[:], in0=F16[:], scalar1=gcol[:],
                                    scalar2=None, op0=Op.mult)
                    v.scalar_tensor_tensor(out=S4[:], in0=Vp[:], scalar=gp1[:],
                                           in1=F16[:], op0=Op.mult,
                                           op1=Op.subtract)
                    dv = smk.tile([P, 1], dt.float32, tag="dv")
                    v.tensor_tensor(out=dv[:], in0=vinit[:], in1=vinit_prev[:],
                                    op=Op.subtract)
                    vice = smk.tile([P, 1], dt.float32, tag="vice")
                    v.scalar_tensor_tensor(out=vice[:], in0=dv[:], scalar=gcol[:],
                                           in1=vinit[:], op0=Op.mult, op1=Op.add)
                    vinit = vice
                # decision blocks: d = (-w[n-1] > D[n]); no vinit needed
                for b in range(NB):
                    lo = b * LB
                    hi = (b + 1) * LB
                    l2 = max(lo, 1)
                    v.scalar_tensor_tensor(out=S5[:, l2:hi],
                                           in0=Vp[:, l2 - 1:hi - 1],
                                           scalar=-1.0, in1=S2[:, l2:hi],
                                           op0=Op.mult, op1=Op.is_gt,
                                           accum_out=sdb[:, b:b + 1])
                # resolve the previous sweep's boundary chain (overlaps the
                # decision blocks above in the DVE queue)
                if pending is not None:
                    resolve_chain()
                # col-0 decision needs winit
                v.scalar_tensor_tensor(out=S5[:, 0:1], in0=vinit[:],
                                       scalar=-1.0, in1=S2[:, 0:1],
                                       op0=Op.mult, op1=Op.is_gt)
                v.tensor_reduce(sd[:], sdb[:], mybir.AxisListType.X, Op.add)
                # fold col-0 decision into the logA bias
                tl = smk.tile([P, 1], dt.float32, tag="tl")
                v.tensor_scalar(out=tl[:], in0=S5[:, 0:1], scalar1=col(9),
                                scalar2=col(8), op0=Op.mult, op1=Op.add)
                A_c, t1 = boundary_A(vinit[:], sd[:], tl[:])
                # a = dA*d + aR with row-start fix via dstar; in-place in S5
                v.tensor_copy(S5[0:1, 0:1], cst[0:1, 12:13])
                v.tensor_copy(S5[64:65, 0:1], cst[64:65, 12:13])
                for b in range(NB):
                    lo = b * LB
                    hi = (b + 1) * LB
                    if b == 0:
                        v.tensor_scalar(out=S5[:, lo:hi], in0=S5[:, lo:hi],
                                        scalar1=col(7), scalar2=col(6),
                                        op0=Op.mult, op1=Op.add)
                    else:
                        s.activation(S5[:, lo:hi], S5[:, lo:hi], Act.Identity,
                                     bias=col(6), scale=col(7))
                    init_ap = vinit[:] if b == 0 else S4[:, lo - 1:lo]
                    v.tensor_tensor_scan(S4[:, lo:hi], S2[:, lo:hi],
                                         S5[:, lo:hi], init_ap,
                                         Op.add, Op.mult)
                # fp16 shadow of w for the E step: keep V_{nsw-3}
                if k == nsw - 3 and "E" in SCHED:
                    s.activation(F16[:], S4[:], Act.Identity, bias=0.0,
                                 scale=1.0)
                pending = (A_c, t1)

            resolve_chain()

            # final exact re-scan (blocked) with POST chained per block
            # S1 = x (prefetched), S2 = D, S3 = x_L -> gain, S5 = a -> y
            NBF = 8
            LBF = L // NBF
            for b in range(NBF):
                lo = b * LBF
                hi = (b + 1) * LBF
                sl = slice(lo, hi)
                init_ap = vinit[:] if b == 0 else S4[:, lo - 1:lo]
                v.tensor_tensor_scan(S4[:, sl], S2[:, sl], S5[:, sl], init_ap,
                                     Op.add, Op.mult)
                # y_L = w + x_L; clip via Relu; gain = exp(-RGAIN*u + col15)
                v.tensor_tensor(out=S5[:, sl], in0=S4[:, sl], in1=S3[:, sl],
                                op=Op.add)
                s.activation(S5[:, sl], S5[:, sl], Act.Relu, bias=col(16),
                             scale=1.0)
                s.activation(S3[:, sl], S5[:, sl], Act.Exp, bias=col(15),
                             scale=-float(RGAIN))
                v.tensor_tensor(out=S5[:, sl], in0=S3[:, sl], in1=S1[:, sl],
                                op=Op.mult)
                nc.sync.dma_start(out=y_out[:, sl], in_=S5[:, sl])

    nc.compile()
    return nc


_CACHE = {}
PROFILE = False
LAST_EXEC_NS = None
LAST_RESULTS = None


def _get_program(L):
    if L not in _CACHE:
        _CACHE[L] = build_program(L)
    return _CACHE[L]


def make_core_inputs(x, params, L):
    """Full x [B,N], params [B,6] -> list of per-core input dicts."""
    B, N = x.shape
    n_cores = B // ROWS_PER_CORE
    rows_per_core = ROWS_PER_CORE
    maxabs = np.abs(x).max(axis=1)
    csts, lR = host_consts(params, maxabs)
    csts[:, 8] = (lR.astype(np.float64) * L).astype(f32)
    ident = np.eye(P, dtype=f32)
    in_maps = []
    for c in range(n_cores):
        rows = slice(c * rows_per_core, (c + 1) * rows_per_core)
        xs = np.ascontiguousarray(x[rows]).reshape(P, L)
        cc = np.repeat(csts[rows], P // rows_per_core, axis=0)
        aux = np.zeros((5, P), f32)
        aux[0, :] = 1.0
        aux[0, 0] = 0.0
        aux[0, 64] = 0.0
        aux[1, 0:64] = 1.0   # Mt row 0 -> partitions 0..63
        aux[2, 64:128] = 1.0
        aux[3, 0] = 1.0      # ones11
        in_maps.append(dict(x=xs, cst=np.ascontiguousarray(cc),
                            aux=aux, ident=ident))
    return in_maps


def kernel(x, params):
    x = np.asarray(x)
    params = np.asarray(params, f32)
    B, N = x.shape
    L = (N * ROWS_PER_CORE) // P
    nc = _get_program(L)
    in_maps = make_core_inputs(np.asarray(x, f32), params, L)
    global LAST_EXEC_NS, LAST_RESULTS
    res = run_bass_kernel_spmd(nc, in_maps, list(range(B // ROWS_PER_CORE)),
                               trace=PROFILE)
    LAST_EXEC_NS = res.exec_time_ns
    LAST_RESULTS = res
    outs = [r["y"].reshape(ROWS_PER_CORE, N) for r in res.results]
    return np.concatenate(outs, axis=0).astype(x.dtype, copy=False)


# revision 12
# speedup vs baseline: 1.3645x; 1.0094x over previous
"""DRC layer (dynamic range compressor) Trainium2 Bass kernel.

Per batch row, y = x * 10^(-y_L/20) * 10^(mk/20) where y_L is a branching
one-pole smoother (attack/release) over the static gain curve x_L
computed in dB domain.  Solved by fixed-point iteration: guess y ->
branch decisions -> recurrence is linear -> solve exactly with the
hardware tensor_tensor_scan -> repeat.  In v = x_L - y space the
recurrence is v[n] = a[n]*(v[n-1] - delta[n]), which is one scan op
(state = (negdelta + state) * a).  Cross-chunk carries are solved
exactly each sweep with a tiny transposed scan over per-chunk affine
maps.  Schedule: 3 sweeps + 1 Aitken-extrapolated sweep (gamma from
boundary-delta ratios), then a final re-scan with exact carries.

vs the 376981ns baseline: 2 fewer sweeps (IIIE vs IIIEII), no DRAM
spill of x_L (V_{k-2} shadow kept in fp16 so SBUF fits), env warm start
pass 1 dropped (chunk-local init), -96 clip moved to the Act engine
(Relu), x re-read by DMA during the last sweep into the dead D buffer.

Sharding: data-parallel, 2 batch rows per core x 8 cores.  Each core
packs its 2 rows as [128, 8192] (partitions 0-63 = row 0 in 64 chunks
of 8192 samples, 64-127 = row 1).
"""
import sys
import numpy as np

try:
    from concourse import bass, bacc, mybir
except Exception:  # pragma: no cover
    for p in ("/opt/trn_rl_repo", "/root/.axon_site/_ro/trn_rl_repo"):
        if p not in sys.path:
            sys.path.insert(0, p)
    from concourse import bass, bacc, mybir

from concourse.bass_utils import run_bass_kernel_spmd
from concourse.tile import TileContext

f32 = np.float32
dt = mybir.dt
Op = mybir.AluOpType
Act = mybir.ActivationFunctionType

SR = f32(44100.0)
LOG9 = float(np.log(9.0))
CL = f32(20.0 / np.log(10.0))       # ln -> dB scale
RGAIN = f32(np.log(10.0) / 20.0)    # dB -> ln scale
P = 128                             # partitions
ROWS_PER_CORE = 2
N_CORES = 8
SCHED = "IIIE"                      # I=sweep, E=extrapolated sweep
NCONST = 17


def host_consts(params, maxabs=None):
    """params [R,6] float32 -> per-row constants [R, NCONST] float32.
    Mirrors the reference's float32 arithmetic for the alphas."""
    p = params.astype(f32)
    p = np.where(np.isnan(p), f32(0.0), p)
    p = np.where(p == 0, f32(1e-10), p)
    T = (-p[:, 0] * f32(60.0)).astype(f32)
    ratio = (p[:, 1] * f32(10.0)).astype(f32)
    attack = np.maximum((p[:, 2] / f32(10.0)).astype(f32), f32(1e-4))
    release = np.maximum((p[:, 3] * f32(3.0)).astype(f32), f32(0.005))
    W = (p[:, 4] * f32(24.0)).astype(f32)
    mk = (p[:, 5] * f32(20.0)).astype(f32)
    aA = np.exp((f32(-LOG9) / (SR * attack)).astype(f32)).astype(f32)
    aR = np.exp((f32(-LOG9) / (SR * release)).astype(f32)).astype(f32)
    # derived (host f64 where it only affects our solver internals)
    lA = np.log(aA.astype(np.float64))
    lR = np.log(aR.astype(np.float64))
    c1 = (1.0 - 1.0 / ratio.astype(np.float64)).astype(f32)
    negc2 = (-1.0 / (8.0 * W.astype(np.float64) * ratio.astype(np.float64))).astype(f32)
    CL64 = np.float64(20.0 / np.log(10.0))
    T64 = T.astype(np.float64); W64 = W.astype(np.float64)
    out = np.zeros((p.shape[0], NCONST), f32)
    out[:, 0] = (-c1.astype(np.float64) * T64).astype(f32)   # negc1T
    out[:, 1] = (c1.astype(np.float64) * CL64 / 2).astype(f32)  # c1*CL/2
    out[:, 2] = negc2
    out[:, 3] = (2 * (W64 / 2 + T64) / CL64).astype(f32)     # thr_above (on u2)
    out[:, 4] = (2 * (T64 - W64 / 2) / CL64).astype(f32)     # thr_below (on u2)
    out[:, 5] = (W64 - T64).astype(f32)                      # W - T (square bias)
    out[:, 6] = aR
    out[:, 7] = aA - aR               # dA
    out[:, 8] = 0.0                   # lRL: filled per-L at call site
    out[:, 9] = (lA - lR).astype(f32)  # dal
    out[:, 10] = (mk.astype(np.float64) * np.log(10.0) / 20.0).astype(f32)  # expbias
    out[:, 11] = 1e-16                # eps2: u2 = ln(x^2 + eps2)
    dA = (aA - aR).astype(np.float64)
    dA = np.where(dA == 0, 1e-30, dA)
    out[:, 12] = ((1.0 - aR.astype(np.float64)) / dA).astype(f32)  # dstar
    # env warm start: aW = aR for fast-attack rows else 0; B = x_L lower bound
    out[:, 13] = np.where(aA < f32(0.99), aR, f32(0.0)).astype(f32)
    if maxabs is None:
        maxabs = np.full(p.shape[0], 1e4)
    uhi = 20.0 * np.log10(np.asarray(maxabs, np.float64) + 1e-8) - T64 + 1.0
    knee_min = -2.25 * (W64 ** 2) / (8.0 * W64 * ratio.astype(np.float64))
    c1f = c1.astype(np.float64)
    B = np.minimum(0.0, knee_min)
    B = np.minimum(B, np.where(c1f < 0, c1f * np.maximum(uhi, 0.0), 0.0)) - 1.0
    out[:, 14] = (-B).astype(f32)                            # negB
    # expbias + 96*RGAIN (post: exp(-RGAIN*relu(96-(-y_L)) + col15))
    out[:, 15] = (out[:, 10].astype(np.float64)
                  + 96.0 * np.log(10.0) / 20.0).astype(f32)
    out[:, 16] = 96.0                                        # relu bias
    out_lR = lR.astype(f32)
    return out, out_lR


def build_program(L):
    """Build the SPMD Bass program for chunk length L (8192 for the real
    problem). Returns the compiled Bacc."""
    nc = bacc.Bacc("TRN2", target_bir_lowering=False, debug=False,
                   num_devices=N_CORES)
    x_in = nc.dram_tensor("x", (P, L), dt.float32, kind="ExternalInput")
    cst_in = nc.dram_tensor("cst", (P, NCONST), dt.float32, kind="ExternalInput")
    aux_in = nc.dram_tensor("aux", (5, P), dt.float32, kind="ExternalInput")
    ident_in = nc.dram_tensor("ident", (P, P), dt.float32, kind="ExternalInput")
    y_out = nc.dram_tensor("y", (P, L), dt.float32, kind="ExternalOutput")

    v = nc.vector
    s = nc.scalar
    te = nc.tensor

    NB = 4                      # col blocks for iter/post pipelining
    LB = L // NB
    NBP = 4                     # pre col blocks
    LBP = L // NBP

    with TileContext(nc) as tc:
        with (
            tc.tile_pool(name="big", bufs=1) as big,
            tc.tile_pool(name="sm", bufs=2) as sm,
            tc.tile_pool(name="smk", bufs=4) as smk,
            tc.tile_pool(name="ps", bufs=1, space="PSUM") as ps,
        ):
            # ---- persistent small tiles (cst DMA'd up front; the
            # boundary-only tiles are DMA'd after the first x block)
            cst = sm.tile([P, NCONST], dt.float32, tag="cst")
            maskt = sm.tile([1, P], dt.float32, tag="maskt")
            mtt = sm.tile([2, P], dt.float32, tag="mtt")
            onest = sm.tile([1, 1], dt.float32, tag="onest")
            ident = sm.tile([P, P], dt.float32, tag="ident")
            startmask = maskt[0:1, :]    # [1,128]: 0 at chunk 0 and 64
            mt = mtt[0:2, :]             # [2,128] row-block indicator
            ones11 = onest[0:1, 0:1]     # [1,1] = 1.0

            def col(i):
                return cst[:, i:i + 1]

            # ---- big slots (32KB/partition each; F16 16KB, MB 16KB)
            S1 = big.tile([P, L], dt.float32, tag="S1")  # sq scratch -> x
            S2 = big.tile([P, L], dt.float32, tag="S2")  # x -> u2 -> D
            S3 = big.tile([P, L], dt.float32, tag="S3")  # x_L -> gain
            S4 = big.tile([P, L], dt.float32, tag="S4")  # stage -> xh -> w
            S5 = big.tile([P, L], dt.float32, tag="S5")  # lin/mask -> d/a -> y
            F16 = big.tile([P, L], dt.float16, tag="F16")  # env, then Vm1 shadow
            MB = big.tile([P, LBP], dt.int32, tag="MB")  # above-mask

            # ====== PRE (w-space): x -> x_L, D; env chained per block ======
            # u2 = ln(x^2+eps2) = 2*ln|x|; all u-space consts pre-doubled.
            # smaller lead blocks shorten the serial Act-chain ramp.
            pre_edges = [0, L // 8, L // 4, L // 2, 3 * L // 4, L]
            for b in range(len(pre_edges) - 1):
                lo = pre_edges[b]
                hi = pre_edges[b + 1]
                sl = slice(lo, hi)
                mb = MB[:, 0:hi - lo]
                nc.sync.dma_start(out=S4[:, sl], in_=x_in[:, sl])
                if b == 0:
                    nc.sync.dma_start(out=cst[:], in_=cst_in[:])
                if b == 1:
                    nc.sync.dma_start(out=maskt[:], in_=aux_in[0:1, :])
                    nc.sync.dma_start(out=mtt[:], in_=aux_in[1:3, :])
                    nc.sync.dma_start(out=onest[:], in_=aux_in[3:4, 0:1])
                    nc.sync.dma_start(out=ident[:], in_=ident_in[:])
                s.activation(S1[:, sl], S4[:, sl], Act.Square, bias=0.0,
                             scale=1.0)
                s.activation(S4[:, sl], S1[:, sl], Act.Ln, bias=col(11),
                             scale=1.0)
                s.activation(S1[:, sl], S4[:, sl], Act.Square, bias=col(5),
                             scale=float(CL / 2))
                s.activation(S3[:, sl], S1[:, sl], Act.Identity, bias=0.0,
                             scale=col(2))
                s.activation(S5[:, sl], S4[:, sl], Act.Identity, bias=col(0),
                             scale=col(1))
                v.tensor_scalar(out=mb, in0=S4[:, sl], scalar1=col(3),
                                scalar2=None, op0=Op.is_gt)
                v.copy_predicated(S3[:, sl], mb, S5[:, sl])
                v.tensor_scalar(out=S5[:, sl], in0=S4[:, sl], scalar1=col(4),
                                scalar2=None, op0=Op.is_ge)
                v.tensor_tensor(out=S3[:, sl], in0=S3[:, sl], in1=S5[:, sl],
                                op=Op.mult)
                # S3 = x_L block.  D = x_L[n-1] - x_L[n] into S2
                v.tensor_tensor(out=S2[:, max(lo, 1):hi],
                                in0=S3[:, max(lo, 1) - 1:hi - 1],
                                in1=S3[:, max(lo, 1):hi], op=Op.subtract)
                # env warm start, chained per block: xh -> S4, aW -> S5,
                # decaying max into F16 (fp16 is plenty for a warm start)
                s.activation(S4[:, sl], S3[:, sl], Act.Identity, bias=col(14),
                             scale=1.0)
                s.activation(S5[:, sl], S3[:, sl], Act.Identity, bias=col(13),
                             scale=0.0)
                env_init = 0.0 if lo == 0 else F16[:, lo - 1:lo]
                v.tensor_tensor_scan(F16[:, sl], S5[:, sl], S4[:, sl],
                                     env_init, Op.mult, Op.max)
            # cross-chunk delta col 0: prevlast[p] = x_L[p-1, L-1], rows reset 0
            pl = smk.tile([P, 1], dt.float32, tag="pl")
            v.memset(pl[:], 0.0)
            nc.sync.dma_start(out=pl[1:P, :], in_=S3[0:P - 1, L - 1:L])
            v.memset(pl[64:65, :], 0.0)
            v.memset(pl[0:1, :], 0.0)
            v.tensor_tensor(out=S2[:, 0:1], in0=pl[:], in1=S3[:, 0:1],
                            op=Op.subtract)
            # x prefetch for the post phase: S1 is free from here on
            for b in range(NB):
                sl = slice(b * LB, (b + 1) * LB)
                nc.sync.dma_start(out=S1[:, sl], in_=x_in[:, sl])

            # ================= iteration machinery =================
            def boundary_A(vinit_used, sd, bias_ap):
                """A-column and A*winit: only needs sum(d); overlaps scans."""
                logA = smk.tile([P, 1], dt.float32, tag="logA")
                v.scalar_tensor_tensor(out=logA[:], in0=sd, scalar=col(9),
                                       in1=bias_ap, op0=Op.mult, op1=Op.add)
                A_c = smk.tile([P, 1], dt.float32, tag="A_c")
                s.activation(A_c[:], logA[:], Act.Exp, bias=0.0, scale=1.0)
                t1 = smk.tile([P, 1], dt.float32, tag="t1")
                v.tensor_tensor(out=t1[:], in0=A_c[:], in1=vinit_used,
                                op=Op.mult)
                return A_c, t1

            def boundary_chain(V_t, A_c, t1):
                f_c = smk.tile([P, 1], dt.float32, tag="f_c")
                v.tensor_tensor(out=f_c[:], in0=V_t[:, L - 1:L], in1=t1[:],
                                op=Op.subtract)
                ap_p = ps.tile([1, P], dt.float32, tag="ap_p")
                te.transpose(ap_p[:], A_c[:], ident[:])
                a_row = smk.tile([1, P], dt.float32, tag="a_row")
                v.tensor_tensor(out=a_row[:], in0=ap_p[:], in1=startmask,
                                op=Op.mult)
                fp_p = ps.tile([1, P], dt.float32, tag="fp_p")
                te.transpose(fp_p[:], f_c[:], ident[:])
                f_row = smk.tile([1, P], dt.float32, tag="f_row")
                v.tensor_copy(f_row[:], fp_p[:])
                zr = smk.tile([1, P], dt.float32, tag="zr")
                v.tensor_tensor_scan(zr[:], a_row[:], f_row[:], 0.0,
                                     Op.mult, Op.add)
                zs = smk.tile([1, P], dt.float32, tag="zs")
                v.memset(zs[:], 0.0)
                v.tensor_copy(zs[0:1, 1:P], zr[0:1, 0:P - 1])
                v.tensor_tensor(out=zs[:], in0=zs[:], in1=startmask, op=Op.mult)
                vip = ps.tile([P, 1], dt.float32, tag="vip")
                te.transpose(vip[:], zs[:], ones11)
                vic = smk.tile([P, 1], dt.float32, tag="vic")
                v.tensor_copy(vic[:], vip[:])
                return vic, zs

            def row_broadcast(pair_row):
                pr = ps.tile([2, 1], dt.float32, tag="pr")
                te.transpose(pr[:], pair_row, ones11)
                prs = smk.tile([2, 1], dt.float32, tag="prs")
                v.tensor_copy(prs[:], pr[:])
                cb = ps.tile([P, 1], dt.float32, tag="cb")
                te.matmul(cb[:], mt, prs[:])
                out = smk.tile([P, 1], dt.float32, tag="bc")
                v.tensor_copy(out[:], cb[:])
                return out

            # w0 = env - xh;  winit0 = shift(w0 chunk ends) masked at rows
            v.tensor_tensor(out=S4[:], in0=F16[:], in1=S4[:], op=Op.subtract)
            vzp = ps.tile([1, P], dt.float32, tag="fp_p")
            te.transpose(vzp[:], S4[:, L - 1:L], ident[:])
            vz_row = smk.tile([1, P], dt.float32, tag="f_row")
            v.tensor_copy(vz_row[:], vzp[:])
            vzs = smk.tile([1, P], dt.float32, tag="d1")
            v.memset(vzs[:], 0.0)
            v.tensor_copy(vzs[0:1, 1:P], vz_row[0:1, 0:P - 1])
            v.tensor_tensor(out=vzs[:], in0=vzs[:], in1=startmask, op=Op.mult)
            vip0 = ps.tile([P, 1], dt.float32, tag="vip")
            te.transpose(vip0[:], vzs[:], ones11)
            vinit0 = smk.tile([P, 1], dt.float32, tag="vic")
            v.tensor_copy(vinit0[:], vip0[:])

            vinit = vinit0
            vinit_prev = None
            zrow_hist = [vzs]
            pending = None          # (A_c, t1) of the un-resolved chain
            nsw = len(SCHED)

            def resolve_chain():
                nonlocal vinit, vinit_prev, pending
                vic, zs = boundary_chain(S4, *pending)
                pending = None
                vinit_prev = vinit
                vinit = vic
                zrow_hist.append(zs)
                if len(zrow_hist) > 3:
                    zrow_hist.pop(0)

            for k, step in enumerate(SCHED):
                sd = smk.tile([P, 1], dt.float32, tag="sd")
                sdb = smk.tile([P, NB], dt.float32, tag="sdb")
                Vp = S4
                if step == "E":
                    # gamma needs z_{k-1}: resolve the chain first
                    if pending is not None:
                        resolve_chain()
                    z0, z1, z2 = zrow_hist[-1], zrow_hist[-2], zrow_hist[-3]
                    d1 = smk.tile([1, P], dt.float32, tag="d1")
                    v.tensor_tensor(out=d1[:], in0=z0[:], in1=z1[:], op=Op.subtract)
                    s.activation(d1[:], d1[:], Act.Abs, bias=0.0, scale=1.0)
                    d2 = smk.tile([1, P], dt.float32, tag="d2")
                    v.tensor_tensor(out=d2[:], in0=z1[:], in1=z2[:], op=Op.subtract)
                    s.activation(d2[:], d2[:], Act.Abs, bias=0.0, scale=1.0)
                    rs = smk.tile([1, 2], dt.float32, tag="rs")
                    rs2 = smk.tile([1, 2], dt.float32, tag="rs2")
                    half = P // 2
                    v.tensor_reduce(rs[0:1, 0:1], d1[0:1, 0:half], mybir.AxisListType.X, Op.add)
                    v.tensor_reduce(rs[0:1, 1:2], d1[0:1, half:P], mybir.AxisListType.X, Op.add)
                    v.tensor_reduce(rs2[0:1, 0:1], d2[0:1, 0:half], mybir.AxisListType.X, Op.add)
                    v.tensor_reduce(rs2[0:1, 1:2], d2[0:1, half:P], mybir.AxisListType.X, Op.add)
                    v.tensor_scalar(out=rs2[:], in0=rs2[:], scalar1=1e-30,
                                    scalar2=None, op0=Op.add)
                    rho = smk.tile([1, 2], dt.float32, tag="rho")
                    v.reciprocal(rs2[:], rs2[:])
                    v.tensor_tensor(out=rho[:], in0=rs[:], in1=rs2[:], op=Op.mult)
                    v.tensor_scalar(out=rho[:], in0=rho[:], scalar1=0.95,
                                    scalar2=None, op0=Op.min)
                    om = smk.tile([1, 2], dt.float32, tag="om")
                    v.tensor_scalar(out=om[:], in0=rho[:], scalar1=-1.0,
                                    scalar2=1.0, op0=Op.mult, op1=Op.add)
                    v.reciprocal(om[:], om[:])
                    gam = smk.tile([1, 2], dt.float32, tag="gam")
                    v.tensor_tensor(out=gam[:], in0=rho[:], in1=om[:], op=Op.mult)
                    gcol = row_broadcast(gam[:])
                    gp1 = smk.tile([P, 1], dt.float32, tag="gp1")
                    v.tensor_scalar(out=gp1[:], in0=gcol[:], scalar1=1.0,
                                    scalar2=None, op0=Op.add)
                    # Ve = (1+g)*Vp - g*Vm1 (fp16 shadow); G=g*Vm1 in place
                    v.tensor_scalar(out=F16[:], in0=F16[:], scalar1=gcol[:],
                                    scalar2=None, op0=Op.mult)
                    v.scalar_tensor_tensor(out=S4[:], in0=Vp[:], scalar=gp1[:],
                                           in1=F16[:], op0=Op.mult,
                                           op1=Op.subtract)
                    dv = smk.tile([P, 1], dt.float32, tag="dv")
                    v.tensor_tensor(out=dv[:], in0=vinit[:], in1=vinit_prev[:],
                                    op=Op.subtract)
                    vice = smk.tile([P, 1], dt.float32, tag="vice")
                    v.scalar_tensor_tensor(out=vice[:], in0=dv[:], scalar=gcol[:],
                                           in1=vinit[:], op0=Op.mult, op1=Op.add)
                    vinit = vice
                # decision blocks: d = (-w[n-1] > D[n]); no vinit needed
                for b in range(NB):
                    lo = b * LB
                    hi = (b + 1) * LB
                    l2 = max(lo, 1)
                    v.scalar_tensor_tensor(out=S5[:, l2:hi],
                                           in0=Vp[:, l2 - 1:hi - 1],
                                           scalar=-1.0, in1=S2[:, l2:hi],
                                           op0=Op.mult, op1=Op.is_gt,
                                           accum_out=sdb[:, b:b + 1])
                # resolve the previous sweep's boundary chain (overlaps the
                # decision blocks above in the DVE queue)
                if pending is not None:
                    resolve_chain()
                # col-0 decision needs winit
                v.scalar_tensor_tensor(out=S5[:, 0:1], in0=vinit[:],
                                       scalar=-1.0, in1=S2[:, 0:1],
                                       op0=Op.mult, op1=Op.is_gt)
                v.tensor_reduce(sd[:], sdb[:], mybir.AxisListType.X, Op.add)
                # fold col-0 decision into the logA bias
                tl = smk.tile([P, 1], dt.float32, tag="tl")
                v.tensor_scalar(out=tl[:], in0=S5[:, 0:1], scalar1=col(9),
                                scalar2=col(8), op0=Op.mult, op1=Op.add)
                A_c, t1 = boundary_A(vinit[:], sd[:], tl[:])
                # a = dA*d + aR with row-start fix via dstar; in-place in S5
                v.tensor_copy(S5[0:1, 0:1], cst[0:1, 12:13])
                v.tensor_copy(S5[64:65, 0:1], cst[64:65, 12:13])
                for b in range(NB):
                    lo = b * LB
                    hi = (b + 1) * LB
                    if b == 0:
                        v.tensor_scalar(out=S5[:, lo:hi], in0=S5[:, lo:hi],
                                        scalar1=col(7), scalar2=col(6),
                                        op0=Op.mult, op1=Op.add)
                    else:
                        s.activation(S5[:, lo:hi], S5[:, lo:hi], Act.Identity,
                                     bias=col(6), scale=col(7))
                    init_ap = vinit[:] if b == 0 else S4[:, lo - 1:lo]
                    v.tensor_tensor_scan(S4[:, lo:hi], S2[:, lo:hi],
                                         S5[:, lo:hi], init_ap,
                                         Op.add, Op.mult)
                # fp16 shadow of w for the E step: keep V_{nsw-3}
                if k == nsw - 3 and "E" in SCHED:
                    s.activation(F16[:], S4[:], Act.Identity, bias=0.0,
                                 scale=1.0)
                pending = (A_c, t1)

            resolve_chain()

            # final exact re-scan (blocked) with POST chained per block
            # S1 = x (prefetched), S2 = D, S3 = x_L -> gain, S5 = a -> y
            NBF = 8
            LBF = L // NBF
            for b in range(NBF):
                lo = b * LBF
                hi = (b + 1) * LBF
                sl = slice(lo, hi)
                init_ap = vinit[:] if b == 0 else S4[:, lo - 1:lo]
                v.tensor_tensor_scan(S4[:, sl], S2[:, sl], S5[:, sl], init_ap,
                                     Op.add, Op.mult)
                # y_L = w + x_L; clip via Relu; gain = exp(-RGAIN*u + col15)
                v.tensor_tensor(out=S5[:, sl], in0=S4[:, sl], in1=S3[:, sl],
                                op=Op.add)
                s.activation(S5[:, sl], S5[:, sl], Act.Relu, bias=col(16),
                             scale=1.0)
                s.activation(S3[:, sl], S5[:, sl], Act.Exp, bias=col(15),
                             scale=-float(RGAIN))
                v.tensor_tensor(out=S5[:, sl], in0=S3[:, sl], in1=S1[:, sl],
                                op=Op.mult)
                nc.sync.dma_start(out=y_out[:, sl], in_=S5[:, sl])

    nc.compile()
    return nc


_CACHE = {}
PROFILE = False
LAST_EXEC_NS = None
LAST_RESULTS = None


def _get_program(L):
    if L not in _CACHE:
        _CACHE[L] = build_program(L)
    return _CACHE[L]


def make_core_inputs(x, params, L):
    """Full x [B,N], params [B,6] -> list of per-core input dicts."""
    B, N = x.shape
    n_cores = B // ROWS_PER_CORE
    rows_per_core = ROWS_PER_CORE
    maxabs = np.abs(x).max(axis=1)
    csts, lR = host_consts(params, maxabs)
    csts[:, 8] = (lR.astype(np.float64) * L).astype(f32)
    ident = np.eye(P, dtype=f32)
    in_maps = []
    for c in range(n_cores):
        rows = slice(c * rows_per_core, (c + 1) * rows_per_core)
        xs = np.ascontiguousarray(x[rows]).reshape(P, L)
        cc = np.repeat(csts[rows], P // rows_per_core, axis=0)
        aux = np.zeros((5, P), f32)
        aux[0, :] = 1.0
        aux[0, 0] = 0.0
        aux[0, 64] = 0.0
        aux[1, 0:64] = 1.0   # Mt row 0 -> partitions 0..63
        aux[2, 64:128] = 1.0
        aux[3, 0] = 1.0      # ones11
        in_maps.append(dict(x=xs, cst=np.ascontiguousarray(cc),
                            aux=aux, ident=ident))
    return in_maps


def kernel(x, params):
    x = np.asarray(x)
    params = np.asarray(params, f32)
    B, N = x.shape
    L = (N * ROWS_PER_CORE) // P
    nc = _get_program(L)
    in_maps = make_core_inputs(np.asarray(x, f32), params, L)
    global LAST_EXEC_NS, LAST_RESULTS
    res = run_bass_kernel_spmd(nc, in_maps, list(range(B // ROWS_PER_CORE)),
                               trace=PROFILE)
    LAST_EXEC_NS = res.exec_time_ns
    LAST_RESULTS = res
    outs = [r["y"].reshape(ROWS_PER_CORE, N) for r in res.results]
    return np.concatenate(outs, axis=0).astype(x.dtype, copy=False)
